# revision 18
# baseline (speedup 1.0000x reference)
"""Multi-head attention (B=4, S=2048, D=1024, H=16) on 8 TRN2 NeuronCores.

Sharding: core c -> (batch b = c//2, head-group g = c%2): each core runs 8
heads of one batch (dout slice of 512) and emits two fp32 out-projection
partials (pairs 0-1 and 2-3); the host sums 4 partials per batch + bias.

All matmul operands are bf16 (fp32 PSUM accumulation); exp runs on the Act
engine (fp32 psum -> bf16); the softmax row-sum is folded into the AV matmul
via a ones-column on v; normalization = DVE reciprocal -> GPSIMD
partition_broadcast -> DVE multiply (no PE involvement). v-projection is
computed directly in transposed [seq, dout] layout (no PE transposes). The
v bias is folded into the host-side output bias (softmax rows sum to 1).

Schedule: k-proj (pairs 0-2) + q-proj(pair0, chunk0) prologue with
interleaved DMA sequencing; 16 attention blocks (pair a, 512-wide query
chunk j) run a depth-2 software pipeline over key-tile PAIRS (step p emits
fillers, QK/exp(p+2), AV(p)); scores for (2p, 2p+1) share one [128,2,512]
psum tile per head so exp stays 1024-wide. PSUM: score ring 2x[128,2,512]
(4 banks) + AV accumulators 2x[65,512] (2 banks) + a DEDICATED filler pool
2x[128,512] (2 banks) — filler matmuls (vT-proj, q/k-proj, out-proj) never
touch the score ring, which keeps its slot-reuse dependencies phase-locked
to one-period-old exps. The last two QK slots of each block prefetch the
next block; out-proj groups depending on the final normalization run as
tail fillers (alt-engine copies hide the norm chain) and a short epilogue.
PE is the critical engine at ~94% busy (~331 us of 354 us total); Act/exp
~272 us, DVE ~112 us, DMA ~105 us, Pool ~26 us all hide under it.
"""
from contextlib import ExitStack

import ml_dtypes
import numpy as np

import concourse.bacc as bacc
import concourse.tile as tile
from concourse import mybir
from concourse.bass_utils import run_bass_kernel_spmd

F32 = mybir.dt.float32
ALU_ADD = mybir.AluOpType.add
BF = mybir.dt.bfloat16
AF = mybir.ActivationFunctionType
NPBF = ml_dtypes.bfloat16

B, S, D, H, HD = 4, 2048, 1024, 16, 64
GS = D // 2            # 512: per-core dout slice (8 heads, 4 pairs)
NP = GS // 128         # 4 head pairs (= dout tiles = wo k-tiles)
NK = D // 128          # 8 din k-tiles
NSK = S // 128         # 16 key tiles
SQ = 512               # query chunk (block width)
NSQ = S // SQ          # 4
NIP = NSK // 2         # 8 key-tile pairs per block
NCH = S // 512         # 4 (512-wide chunks of S)

_CACHE = {}


def _build_nc():
    if "nc" in _CACHE:
        return _CACHE["nc"]

    nc = bacc.Bacc()

    xqT = nc.dram_tensor("xqT", [128, NK, S], BF, kind="ExternalInput")
    xkT = nc.dram_tensor("xkT", [128, NK, S], BF, kind="ExternalInput")
    xvT = nc.dram_tensor("xvT", [128, NK, S], BF, kind="ExternalInput")
    wqT = nc.dram_tensor("wqT", [128, NK, GS], BF, kind="ExternalInput")
    wkT = nc.dram_tensor("wkT", [128, NK, GS], BF, kind="ExternalInput")
    wvT = nc.dram_tensor("wvT", [128, NK, GS], BF, kind="ExternalInput")
    woT = nc.dram_tensor("woT", [128, NP, D], BF, kind="ExternalInput")
    biasqk = nc.dram_tensor("biasqk", [128, 8], F32, kind="ExternalInput")
    outTs = [nc.dram_tensor(f"outT{p}", [128, NK, S], F32,
                            kind="ExternalOutput") for p in range(2)]

    with tile.TileContext(nc) as tc, ExitStack() as kctx:
        consts = kctx.enter_context(tc.tile_pool(name="consts", bufs=1))
        pool_w = kctx.enter_context(tc.tile_pool(name="wp", bufs=1))
        pool_xq = kctx.enter_context(tc.tile_pool(name="xqp", bufs=1))
        pool_xs = kctx.enter_context(tc.tile_pool(name="xsp", bufs=3))
        pool_k = kctx.enter_context(tc.tile_pool(name="kTp", bufs=1))
        pool_q = kctx.enter_context(tc.tile_pool(name="qTp", bufs=4))
        pool_va = kctx.enter_context(tc.tile_pool(name="vap", bufs=1))
        pool_e = kctx.enter_context(tc.tile_pool(name="ep", bufs=8))
        pool_oT = kctx.enter_context(tc.tile_pool(name="oTp", bufs=1))
        pool_rr = kctx.enter_context(tc.tile_pool(name="rrp", bufs=3))
        pool_rb = kctx.enter_context(tc.tile_pool(name="rbp", bufs=3))
        pool_oo = kctx.enter_context(tc.tile_pool(name="oop", bufs=12))
        pp_qk = kctx.enter_context(tc.tile_pool(name="ppqk", bufs=2,
                                                space="PSUM"))
        pp_av = kctx.enter_context(tc.tile_pool(name="ppav", bufs=2,
                                                space="PSUM"))
        pp_fill = kctx.enter_context(tc.tile_pool(name="ppfl", bufs=2,
                                                  space="PSUM"))

        bias_t = consts.tile([128, 8], F32)

        # ---------------- static SBUF tensors ----------------
        wk_t = pool_w.tile([128, NK, GS], BF, name="wk")
        wq_t = pool_w.tile([128, NK, GS], BF, name="wq")
        wv_t = pool_w.tile([128, NK, GS], BF, name="wv")
        wo_t = pool_w.tile([128, NP, D], BF, name="wo")
        xq_t = pool_xq.tile([128, NK, S], BF, name="xq")
        kT = [pool_k.tile([128, S], BF, name=f"kT{m}") for m in range(NP)]
        v_aug = [pool_va.tile([128, 8, HD + 1], BF, name=f"va{i}")
                 for i in range(NSK)]
        o_tiles = [pool_oT.tile([128, S], BF, name=f"oT{a}")
                   for a in range(NP)]
        q_tiles = {}

        # ---------------- prologue: k-proj (all pairs) ----------------
        nc.sync.dma_start(out=wk_t[:, :, 0:256], in_=wkT[:, :, 0:256])
        xk_tiles = {}

        def xk_dma(n, split=False):
            xk_tiles[n] = pool_xs.tile([128, NK, 512], BF, tag="xs",
                                       name=f"xk{n}")
            if split:
                for q in range(4):
                    nc.sync.dma_start(
                        out=xk_tiles[n][:, 2 * q:2 * q + 2, :],
                        in_=xkT[:, 2 * q:2 * q + 2,
                                n * 512:(n + 1) * 512])
            else:
                nc.sync.dma_start(out=xk_tiles[n],
                                  in_=xkT[:, :, n * 512:(n + 1) * 512])

        def kproj_group(m, n, tiles):
            ps = pp_fill.tile([128, 512], F32, tag="fl", name=f"psk{n}{m}")
            for kk in range(NK):
                nc.tensor.matmul(
                    ps[:],
                    wk_t[:, kk, m * 128:(m + 1) * 128],
                    tiles[n][:, kk, :],
                    start=(kk == 0),
                    stop=(kk == NK - 1),
                )
            nc.vector.tensor_scalar_add(
                kT[m][:, n * 512:(n + 1) * 512], ps[:],
                bias_t[:, 4 + m:5 + m])

        # prologue covers pairs 0-2; pair 3 runs as mid-span fillers
        xk_dma(0, split=True)
        nc.sync.dma_start(out=bias_t, in_=biasqk[:, :])
        nc.sync.dma_start(out=wk_t[:, :, 256:512], in_=wkT[:, :, 256:512])
        xk_dma(1)
        for n in range(NCH):
            if n + 2 < NCH:
                xk_dma(n + 2)
            if n == 2:
                nc.sync.dma_start(out=wq_t, in_=wqT[:, :, :])
            if n == 3:
                nc.sync.dma_start(out=wv_t, in_=wvT[:, :, :])
            for m in range(3):
                kproj_group(m, n, xk_tiles)

        def kproj_fillers(m):
            """4 filler groups for k-proj of pair m (re-streams xk)."""
            tiles = {}

            def dma(n):
                tiles[n] = pool_xs.tile([128, NK, 512], BF, tag="xs",
                                        name=f"xk{m}_{n}")
                nc.sync.dma_start(out=tiles[n],
                                  in_=xkT[:, :, n * 512:(n + 1) * 512])

            def group(n):
                def run():
                    if n + 1 < NCH:
                        dma(n + 1)
                    kproj_group(m, n, tiles)
                return run
            return [group(n) for n in range(NCH)], dma

        def xq_dma(n):
            nc.sync.dma_start(out=xq_t[:, :, n * 512:(n + 1) * 512],
                              in_=xqT[:, :, n * 512:(n + 1) * 512])

        def qproj_groups(a):
            qt = pool_q.tile([128, S], BF, tag="qT", name=f"qT{a}")
            q_tiles[a] = qt

            def group(n):
                def run():
                    ps = pp_fill.tile([128, 512], F32, tag="fl",
                                     name=f"psq{a}{n}")
                    for kk in range(NK):
                        nc.tensor.matmul(
                            ps[:],
                            wq_t[:, kk, a * 128:(a + 1) * 128],
                            xq_t[:, kk, n * 512:(n + 1) * 512],
                            start=(kk == 0),
                            stop=(kk == NK - 1),
                        )
                    nc.vector.tensor_scalar_add(
                        qt[:, n * 512:(n + 1) * 512], ps[:],
                        bias_t[:, a:a + 1])
                return run
            return [group(n) for n in range(NCH)]

        q0 = qproj_groups(0)

        # ---------------- vT-proj groups (one per seq-tile st) ----------
        xv_tiles = {}

        def xv_dma(n):
            xv_tiles[n] = pool_xs.tile([128, NK, 512], BF, tag="xs",
                                       name=f"xv{n}")
            nc.sync.dma_start(out=xv_tiles[n],
                              in_=xvT[:, :, n * 512:(n + 1) * 512])

        def vt_group(st):
            def run():
                n, sl = st // 4, st % 4
                if sl == 0 and 1 <= n < NCH - 1:
                    xv_dma(n + 1)
                ps = pp_fill.tile([128, 512], F32, tag="fl", name=f"psv{st}")
                for kk in range(NK):
                    nc.tensor.matmul(
                        ps[:],
                        xv_tiles[n][:, kk, sl * 128:(sl + 1) * 128],
                        wv_t[:, kk, :],
                        start=(kk == 0),
                        stop=(kk == NK - 1),
                    )
                nc.vector.memset(v_aug[st][:, :, HD:HD + 1], 1.0)
                nc.vector.tensor_copy(v_aug[st][:, :, 0:HD], ps[:])
            return run

        vt_fill = [vt_group(st) for st in range(NSK)]
        xq_dma(0)
        xv_dma(0)
        q0[0]()
        xq_dma(1)
        vt_fill[0]()
        xv_dma(1)
        vt_fill[1]()
        xq_dma(2)
        xq_dma(3)

        nc.sync.dma_start(out=wo_t, in_=woT[:, :, :])

        # ---------------- out-proj groups ----------------
        def outproj_groups(p, jjs, copy_eng="vector"):
            def group(dm, jj, gi):
                def run():
                    ps = pp_fill.tile([128, 512], F32, tag="fl",
                                     name=f"pso{p}{dm}{jj}")
                    for a in (2 * p, 2 * p + 1):
                        nc.tensor.matmul(
                            ps[:],
                            wo_t[:, a, dm * 128:(dm + 1) * 128],
                            o_tiles[a][:, jj * 512:(jj + 1) * 512],
                            start=(a == 2 * p),
                            stop=(a == 2 * p + 1),
                        )
                    oo = pool_oo.tile([128, 512], F32, tag="oo",
                                      name=f"oo{p}{dm}{jj}")
                    use_act = (copy_eng == "scalar"
                               or (copy_eng == "alt" and gi % 2))
                    if use_act:
                        nc.scalar.copy(oo[:], ps[:])
                    else:
                        nc.vector.tensor_copy(oo[:], ps[:])
                    nc.sync.dma_start(
                        out=outTs[p][:, dm, jj * 512:(jj + 1) * 512],
                        in_=oo[:])
                return run
            return [group(dm, jj, gi)
                    for gi, (jj, dm) in enumerate(
                        (jj, dm) for jj in jjs for dm in range(NK))]

        # ---------------- attention ----------------
        def make_qk_exp(a, j, ee):
            def qk_exp(p):
                for h in range(2):
                    hb = h * HD
                    sc = pp_qk.tile([128, 2, 512], F32, tag="sc",
                                    name=f"sc{a}{j}{p}{h}")
                    for u in range(2):
                        i = 2 * p + u
                        nc.tensor.matmul(
                            sc[:, u, :],
                            kT[a][hb:hb + HD, i * 128:(i + 1) * 128],
                            q_tiles[a][hb:hb + HD,
                                       j * SQ:(j + 1) * SQ],
                            start=True,
                            stop=True,
                        )
                    e = pool_e.tile([128, 2, 512], BF, tag="e",
                                    name=f"e{a}{j}{p}{h}")
                    nc.scalar.activation(e[:], sc[:], AF.Exp)
                    ee[(p, h)] = e
            return qk_exp

        def attn_block(a, j, fillers, pre, nxt, pace_off=1,
                       tail_fillers=()):
            """Depth-2 software pipeline over key-tile PAIRS: step p emits
            [fillers, QK/exp(p+2), AV(p)]. Scores for (2p, 2p+1) share one
            [128,2,512] psum tile per head so exp stays at 1024-wide; filler
            matmuls use their own pp_fill pool and never touch the score
            ring. The last two QK slots prefetch the next block."""
            po = [pp_av.tile([HD + 1, SQ], F32, tag="po",
                             name=f"po{a}{j}{h}") for h in range(2)]
            ee = pre if pre is not None else {}
            qk_exp = make_qk_exp(a, j, ee)
            if pre is None:
                qk_exp(0)
                qk_exp(1)
            nee = {}
            nqk = make_qk_exp(nxt[0], nxt[1], nee) if nxt else None
            nf, fi = len(fillers), 0
            for p in range(NIP):
                want = (p + pace_off) * nf // NIP
                while fi < min(want, nf):
                    fillers[fi]()
                    fi += 1
                if p + 2 < NIP:
                    qk_exp(p + 2)
                elif nqk is not None:
                    nqk(p + 2 - NIP)
                for h in range(2):
                    e = ee.pop((p, h))
                    for u in range(2):
                        i = 2 * p + u
                        nc.tensor.matmul(
                            po[h][:],
                            v_aug[i][:, 2 * a + h, :],
                            e[:, u, :],
                            start=(i == 0),
                            stop=(i == NSK - 1),
                        )
            while fi < nf:
                fillers[fi]()
                fi += 1
            for g in tail_fillers:
                g()
            # normalization: 1/rowsum broadcast (Pool) and multiply (DVE);
            # recips first so the h0/h1 chains overlap across engines
            with nc.allow_low_precision(reason="bf16 softmax reciprocal"):
                rrs, rbs = [], []
                for h in range(2):
                    rr = pool_rr.tile([1, SQ], BF, tag="rr",
                                      name=f"rr{a}{j}{h}")
                    nc.vector.reciprocal(rr[:], po[h][HD:HD + 1, :])
                    rrs.append(rr)
                for h in range(2):
                    rb = pool_rb.tile([HD, SQ], BF, tag="rb",
                                      name=f"rb{a}{j}{h}")
                    nc.gpsimd.partition_broadcast(rb[:], rrs[h][:])
                    rbs.append(rb)
                for h in range(2):
                    hb = h * HD
                    nc.vector.tensor_mul(
                        o_tiles[a][hb:hb + HD, j * SQ:(j + 1) * SQ],
                        po[h][0:HD, :],
                        rbs[h][:])
            return nee if nxt else None

        q1 = qproj_groups(1)
        q2 = qproj_groups(2)
        q3 = qproj_groups(3)
        k3, k3_dma = kproj_fillers(3)
        op01 = outproj_groups(0, (0, 1, 2, 3))
        op23_0 = outproj_groups(1, (0,))
        op23_1 = outproj_groups(1, (1,))
        op23_2t = outproj_groups(1, (2,), copy_eng="alt")

        plan = [
            (0, 0, vt_fill[2:] + [q0[1]], 3, ()),
            (0, 1, [q0[2], q1[0]], 1, ()),
            (0, 2, [q0[3], q1[1]], 1, ()),
            (0, 3, [q1[2], q1[3], lambda: k3_dma(0)], 1, ()),
            (1, 0, [k3[0], q2[0]], 1, ()),
            (1, 1, [k3[1], q2[1]], 1, ()),
            (1, 2, [k3[2], q2[2]], 1, ()),
            (1, 3, [k3[3], q2[3]], 1, ()),
            (2, 0, [q3[0]] + op01[0:4], 1, ()),
            (2, 1, [q3[1]] + op01[4:8], 1, ()),
            (2, 2, [q3[2]] + op01[8:12], 1, ()),
            (2, 3, [q3[3]] + op01[12:16], 1, ()),
            (3, 0, op01[16:24], 1, ()),
            (3, 1, op01[24:32], 1, ()),
            (3, 2, op23_0, 1, ()),
            # tail fillers (alt copies) hide the final norm chain
            (3, 3, op23_1, 1, op23_2t),
        ]
        pre = None
        for bi, (a, j, fillers, off, tails) in enumerate(plan):
            nxt = plan[bi + 1][0:2] if bi + 1 < len(plan) else None
            pre = attn_block(a, j, fillers, pre, nxt, pace_off=off,
                             tail_fillers=tails)
        for g in outproj_groups(1, (3,), copy_eng="alt"):
            g()

    nc.compile()
    _CACHE["nc"] = nc
    return nc


def _tox(a):
    """[1024|512, N] -> [128, k, N] bf16 (partition-major k-tiling)."""
    r = a.shape[0] // 128
    return np.ascontiguousarray(
        a.reshape(r, 128, a.shape[1]).transpose(1, 0, 2)).astype(NPBF)


def kernel(Q, K, V, Wq, bq, Wk, bk, Wv, bv, Wo, bo):
    Q = np.asarray(Q, np.float32)
    K = np.asarray(K, np.float32)
    V = np.asarray(V, np.float32)
    Wq = np.asarray(Wq, np.float32)
    Wk = np.asarray(Wk, np.float32)
    Wv = np.asarray(Wv, np.float32)
    Wo = np.asarray(Wo, np.float32)
    bq = np.asarray(bq, np.float32)
    bk = np.asarray(bk, np.float32)
    bv = np.asarray(bv, np.float32)
    bo = np.asarray(bo, np.float32)
    scale = 1.0 / 8.0  # 1/sqrt(HD), folded into the q projection

    nc = _build_nc()
    in_maps = []
    for c in range(8):
        b, g = divmod(c, 2)
        gs = slice(g * GS, (g + 1) * GS)
        biasqk = np.empty((128, 8), np.float32)
        for m in range(NP):
            biasqk[:, m] = bq[gs][m * 128:(m + 1) * 128] * scale
            biasqk[:, 4 + m] = bk[gs][m * 128:(m + 1) * 128]
        in_maps.append({
            "xqT": _tox(Q[b].T),
            "xkT": _tox(K[b].T),
            "xvT": _tox(V[b].T),
            "wqT": _tox((Wq[gs] * scale).T),
            "wkT": _tox(Wk[gs].T),
            "wvT": _tox(Wv[gs].T),
            "woT": _tox(Wo[:, gs].T),
            "biasqk": biasqk,
        })

    host_bias = bo + Wo @ bv  # v bias folded through softmax + out-proj

    def run_and_gather():
        res = run_bass_kernel_spmd(nc, in_maps, list(range(8)))
        out = np.empty((B, S, D), np.float32)
        for b in range(B):
            acc = None
            for c in (2 * b, 2 * b + 1):
                for p in range(2):
                    part = np.asarray(res.results[c][f"outT{p}"])
                    part = part.transpose(1, 0, 2).reshape(D, S)
                    acc = part if acc is None else acc + part
            out[b] = acc.T + host_bias
        return out

    try:
        return run_and_gather()
    except Exception:
        # transient device wedge (e.g. NRT_EXEC_UNIT_UNRECOVERABLE) can
        # surface either in the run or in result materialization: retry once
        return run_and_gather()


# revision 20
# speedup vs baseline: 1.0022x; 1.0022x over previous
"""Multi-head attention (B=4, S=2048, D=1024, H=16) on 8 TRN2 NeuronCores.

Sharding: core c -> (batch b = c//2, head-group g = c%2): each core runs 8
heads of one batch (dout slice of 512) and emits two fp32 out-projection
partials (pairs 0-1 and 2-3); the host sums 4 partials per batch + bias.

All matmul operands are bf16 (fp32 PSUM accumulation); exp runs on the Act
engine (fp32 psum -> bf16); the softmax row-sum is folded into the AV matmul
via a ones-column on v; normalization = DVE reciprocal -> GPSIMD
partition_broadcast -> DVE multiply (no PE involvement). v-projection is
computed directly in transposed [seq, dout] layout (no PE transposes). The
v bias is folded into the host-side output bias (softmax rows sum to 1).

Schedule: k-proj (pairs 0-2) + q-proj(pair0, chunk0) prologue with
interleaved DMA sequencing; 16 attention blocks (pair a, 512-wide query
chunk j) run a depth-2 software pipeline over key-tile PAIRS (step p emits
fillers, QK/exp(p+2), AV(p)); scores for (2p, 2p+1) share one [128,2,512]
psum tile per head so exp stays 1024-wide. PSUM: score ring 2x[128,2,512]
(4 banks) + AV accumulators 2x[65,512] (2 banks) + a DEDICATED filler pool
2x[128,512] (2 banks) — filler matmuls (vT-proj, q/k-proj, out-proj) never
touch the score ring, which keeps its slot-reuse dependencies phase-locked
to one-period-old exps. The last two QK slots of each block prefetch the
next block; out-proj groups depending on the final normalization run as
tail fillers (alt-engine copies hide the norm chain) and a short epilogue.
PE is the critical engine at ~94% busy (~331 us of 354 us total); Act/exp
~272 us, DVE ~112 us, DMA ~105 us, Pool ~26 us all hide under it.
"""
from contextlib import ExitStack

import ml_dtypes
import numpy as np

import concourse.bacc as bacc
import concourse.tile as tile
from concourse import mybir
from concourse.bass_utils import run_bass_kernel_spmd

F32 = mybir.dt.float32
F16 = mybir.dt.float16
ALU_ADD = mybir.AluOpType.add
BF = mybir.dt.bfloat16
AF = mybir.ActivationFunctionType
NPBF = ml_dtypes.bfloat16

B, S, D, H, HD = 4, 2048, 1024, 16, 64
GS = D // 2            # 512: per-core dout slice (8 heads, 4 pairs)
NP = GS // 128         # 4 head pairs (= dout tiles = wo k-tiles)
NK = D // 128          # 8 din k-tiles
NSK = S // 128         # 16 key tiles
SQ = 512               # query chunk (block width)
NSQ = S // SQ          # 4
NIP = NSK // 2         # 8 key-tile pairs per block
NCH = S // 512         # 4 (512-wide chunks of S)

_CACHE = {}


def _build_nc():
    if "nc" in _CACHE:
        return _CACHE["nc"]

    nc = bacc.Bacc()

    xqT = nc.dram_tensor("xqT", [128, NK, S], BF, kind="ExternalInput")
    xkT = nc.dram_tensor("xkT", [128, NK, S], BF, kind="ExternalInput")
    xvT = nc.dram_tensor("xvT", [128, NK, S], BF, kind="ExternalInput")
    wqT = nc.dram_tensor("wqT", [128, NK, GS], BF, kind="ExternalInput")
    wkT = nc.dram_tensor("wkT", [128, NK, GS], BF, kind="ExternalInput")
    wvT = nc.dram_tensor("wvT", [128, NK, GS], BF, kind="ExternalInput")
    woT = nc.dram_tensor("woT", [128, NP, D], BF, kind="ExternalInput")
    biasqk = nc.dram_tensor("biasqk", [128, 8], F32, kind="ExternalInput")
    outTs = [nc.dram_tensor(f"outT{p}", [128, NK, S], F16,
                            kind="ExternalOutput") for p in range(2)]

    with tile.TileContext(nc) as tc, ExitStack() as kctx:
        consts = kctx.enter_context(tc.tile_pool(name="consts", bufs=1))
        pool_w = kctx.enter_context(tc.tile_pool(name="wp", bufs=1))
        pool_xq = kctx.enter_context(tc.tile_pool(name="xqp", bufs=1))
        pool_xs = kctx.enter_context(tc.tile_pool(name="xsp", bufs=3))
        pool_k = kctx.enter_context(tc.tile_pool(name="kTp", bufs=1))
        pool_q = kctx.enter_context(tc.tile_pool(name="qTp", bufs=4))
        pool_va = kctx.enter_context(tc.tile_pool(name="vap", bufs=1))
        pool_e = kctx.enter_context(tc.tile_pool(name="ep", bufs=8))
        pool_oT = kctx.enter_context(tc.tile_pool(name="oTp", bufs=1))
        pool_rr = kctx.enter_context(tc.tile_pool(name="rrp", bufs=3))
        pool_rb = kctx.enter_context(tc.tile_pool(name="rbp", bufs=3))
        pool_oo = kctx.enter_context(tc.tile_pool(name="oop", bufs=12))
        pp_qk = kctx.enter_context(tc.tile_pool(name="ppqk", bufs=2,
                                                space="PSUM"))
        pp_av = kctx.enter_context(tc.tile_pool(name="ppav", bufs=2,
                                                space="PSUM"))
        pp_fill = kctx.enter_context(tc.tile_pool(name="ppfl", bufs=2,
                                                  space="PSUM"))

        bias_t = consts.tile([128, 8], F32)

        # ---------------- static SBUF tensors ----------------
        wk_t = pool_w.tile([128, NK, GS], BF, name="wk")
        wq_t = pool_w.tile([128, NK, GS], BF, name="wq")
        wv_t = pool_w.tile([128, NK, GS], BF, name="wv")
        wo_t = pool_w.tile([128, NP, D], BF, name="wo")
        xq_t = pool_xq.tile([128, NK, S], BF, name="xq")
        kT = [pool_k.tile([128, S], BF, name=f"kT{m}") for m in range(NP)]
        v_aug = [pool_va.tile([128, 8, HD + 1], BF, name=f"va{i}")
                 for i in range(NSK)]
        o_tiles = [pool_oT.tile([128, S], BF, name=f"oT{a}")
                   for a in range(NP)]
        q_tiles = {}

        # ---------------- prologue: k-proj (all pairs) ----------------
        nc.sync.dma_start(out=wk_t[:, :, 0:256], in_=wkT[:, :, 0:256])
        xk_tiles = {}

        def xk_dma(n, split=False):
            xk_tiles[n] = pool_xs.tile([128, NK, 512], BF, tag="xs",
                                       name=f"xk{n}")
            if split:
                for q in range(4):
                    nc.sync.dma_start(
                        out=xk_tiles[n][:, 2 * q:2 * q + 2, :],
                        in_=xkT[:, 2 * q:2 * q + 2,
                                n * 512:(n + 1) * 512])
            else:
                nc.sync.dma_start(out=xk_tiles[n],
                                  in_=xkT[:, :, n * 512:(n + 1) * 512])

        def kproj_group(m, n, tiles):
            ps = pp_fill.tile([128, 512], F32, tag="fl", name=f"psk{n}{m}")
            for kk in range(NK):
                nc.tensor.matmul(
                    ps[:],
                    wk_t[:, kk, m * 128:(m + 1) * 128],
                    tiles[n][:, kk, :],
                    start=(kk == 0),
                    stop=(kk == NK - 1),
                )
            nc.vector.tensor_scalar_add(
                kT[m][:, n * 512:(n + 1) * 512], ps[:],
                bias_t[:, 4 + m:5 + m])

        # prologue covers pairs 0-2; pair 3 runs as mid-span fillers
        xk_dma(0, split=True)
        nc.sync.dma_start(out=bias_t, in_=biasqk[:, :])
        nc.sync.dma_start(out=wk_t[:, :, 256:512], in_=wkT[:, :, 256:512])
        xk_dma(1)
        for n in range(NCH):
            if n + 2 < NCH:
                xk_dma(n + 2)
            if n == 2:
                nc.sync.dma_start(out=wq_t, in_=wqT[:, :, :])
            if n == 3:
                nc.sync.dma_start(out=wv_t, in_=wvT[:, :, :])
            for m in range(3):
                kproj_group(m, n, xk_tiles)

        def kproj_fillers(m):
            """4 filler groups for k-proj of pair m (re-streams xk)."""
            tiles = {}

            def dma(n):
                tiles[n] = pool_xs.tile([128, NK, 512], BF, tag="xs",
                                        name=f"xk{m}_{n}")
                nc.sync.dma_start(out=tiles[n],
                                  in_=xkT[:, :, n * 512:(n + 1) * 512])

            def group(n):
                def run():
                    if n + 1 < NCH:
                        dma(n + 1)
                    kproj_group(m, n, tiles)
                return run
            return [group(n) for n in range(NCH)], dma

        def xq_dma(n):
            nc.sync.dma_start(out=xq_t[:, :, n * 512:(n + 1) * 512],
                              in_=xqT[:, :, n * 512:(n + 1) * 512])

        def qproj_groups(a):
            qt = pool_q.tile([128, S], BF, tag="qT", name=f"qT{a}")
            q_tiles[a] = qt

            def group(n):
                def run():
                    ps = pp_fill.tile([128, 512], F32, tag="fl",
                                     name=f"psq{a}{n}")
                    for kk in range(NK):
                        nc.tensor.matmul(
                            ps[:],
                            wq_t[:, kk, a * 128:(a + 1) * 128],
                            xq_t[:, kk, n * 512:(n + 1) * 512],
                            start=(kk == 0),
                            stop=(kk == NK - 1),
                        )
                    nc.vector.tensor_scalar_add(
                        qt[:, n * 512:(n + 1) * 512], ps[:],
                        bias_t[:, a:a + 1])
                return run
            return [group(n) for n in range(NCH)]

        q0 = qproj_groups(0)

        # ---------------- vT-proj groups (one per seq-tile st) ----------
        xv_tiles = {}

        def xv_dma(n):
            xv_tiles[n] = pool_xs.tile([128, NK, 512], BF, tag="xs",
                                       name=f"xv{n}")
            nc.sync.dma_start(out=xv_tiles[n],
                              in_=xvT[:, :, n * 512:(n + 1) * 512])

        def vt_group(st):
            def run():
                n, sl = st // 4, st % 4
                if sl == 0 and 1 <= n < NCH - 1:
                    xv_dma(n + 1)
                ps = pp_fill.tile([128, 512], F32, tag="fl", name=f"psv{st}")
                for kk in range(NK):
                    nc.tensor.matmul(
                        ps[:],
                        xv_tiles[n][:, kk, sl * 128:(sl + 1) * 128],
                        wv_t[:, kk, :],
                        start=(kk == 0),
                        stop=(kk == NK - 1),
                    )
                nc.vector.memset(v_aug[st][:, :, HD:HD + 1], 1.0)
                nc.vector.tensor_copy(v_aug[st][:, :, 0:HD], ps[:])
            return run

        vt_fill = [vt_group(st) for st in range(NSK)]
        xq_dma(0)
        xv_dma(0)
        q0[0]()
        xq_dma(1)
        vt_fill[0]()
        xv_dma(1)
        vt_fill[1]()
        xq_dma(2)
        xq_dma(3)

        nc.sync.dma_start(out=wo_t, in_=woT[:, :, :])

        # ---------------- out-proj groups ----------------
        def outproj_groups(p, jjs, copy_eng="vector", pool_alt=False):
            def group(dm, jj, gi):
                def run():
                    if pool_alt and gi % 2:
                        # post-exp groups: the score ring is idle, borrow its
                        # slots to double the effective psum ring depth
                        ps = pp_qk.tile([128, 512], F32, tag="sc",
                                        name=f"pso{p}{dm}{jj}")
                    else:
                        ps = pp_fill.tile([128, 512], F32, tag="fl",
                                         name=f"pso{p}{dm}{jj}")
                    for a in (2 * p, 2 * p + 1):
                        nc.tensor.matmul(
                            ps[:],
                            wo_t[:, a, dm * 128:(dm + 1) * 128],
                            o_tiles[a][:, jj * 512:(jj + 1) * 512],
                            start=(a == 2 * p),
                            stop=(a == 2 * p + 1),
                        )
                    oo = pool_oo.tile([128, 512], F16, tag="oo",
                                      name=f"oo{p}{dm}{jj}")
                    use_act = (copy_eng == "scalar"
                               or (copy_eng == "alt" and gi % 2))
                    with nc.allow_low_precision(reason="fp16 partial out"):
                        if use_act:
                            nc.scalar.copy(oo[:], ps[:])
                        else:
                            nc.vector.tensor_copy(oo[:], ps[:])
                    nc.sync.dma_start(
                        out=outTs[p][:, dm, jj * 512:(jj + 1) * 512],
                        in_=oo[:])
                return run
            return [group(dm, jj, gi)
                    for gi, (jj, dm) in enumerate(
                        (jj, dm) for jj in jjs for dm in range(NK))]

        # ---------------- attention ----------------
        def make_qk_exp(a, j, ee):
            def qk_exp(p):
                for h in range(2):
                    hb = h * HD
                    sc = pp_qk.tile([128, 2, 512], F32, tag="sc",
                                    name=f"sc{a}{j}{p}{h}")
                    for u in range(2):
                        i = 2 * p + u
                        nc.tensor.matmul(
                            sc[:, u, :],
                            kT[a][hb:hb + HD, i * 128:(i + 1) * 128],
                            q_tiles[a][hb:hb + HD,
                                       j * SQ:(j + 1) * SQ],
                            start=True,
                            stop=True,
                        )
                    e = pool_e.tile([128, 2, 512], BF, tag="e",
                                    name=f"e{a}{j}{p}{h}")
                    nc.scalar.activation(e[:], sc[:], AF.Exp)
                    ee[(p, h)] = e
            return qk_exp

        def attn_block(a, j, fillers, pre, nxt, pace_off=1,
                       tail_fillers=()):
            """Depth-2 software pipeline over key-tile PAIRS: step p emits
            [fillers, QK/exp(p+2), AV(p)]. Scores for (2p, 2p+1) share one
            [128,2,512] psum tile per head so exp stays at 1024-wide; filler
            matmuls use their own pp_fill pool and never touch the score
            ring. The last two QK slots prefetch the next block."""
            po = [pp_av.tile([HD + 1, SQ], F32, tag="po",
                             name=f"po{a}{j}{h}") for h in range(2)]
            ee = pre if pre is not None else {}
            qk_exp = make_qk_exp(a, j, ee)
            if pre is None:
                qk_exp(0)
                qk_exp(1)
            nee = {}
            nqk = make_qk_exp(nxt[0], nxt[1], nee) if nxt else None
            nf, fi = len(fillers), 0
            for p in range(NIP):
                want = (p + pace_off) * nf // NIP
                while fi < min(want, nf):
                    fillers[fi]()
                    fi += 1
                if p + 2 < NIP:
                    qk_exp(p + 2)
                elif nqk is not None:
                    nqk(p + 2 - NIP)
                for h in range(2):
                    e = ee.pop((p, h))
                    for u in range(2):
                        i = 2 * p + u
                        nc.tensor.matmul(
                            po[h][:],
                            v_aug[i][:, 2 * a + h, :],
                            e[:, u, :],
                            start=(i == 0),
                            stop=(i == NSK - 1),
                        )
            while fi < nf:
                fillers[fi]()
                fi += 1
            for g in tail_fillers:
                g()
            # normalization: 1/rowsum broadcast (Pool) and multiply (DVE);
            # recips first so the h0/h1 chains overlap across engines
            with nc.allow_low_precision(reason="bf16 softmax reciprocal"):
                rrs, rbs = [], []
                for h in range(2):
                    rr = pool_rr.tile([1, SQ], BF, tag="rr",
                                      name=f"rr{a}{j}{h}")
                    nc.vector.reciprocal(rr[:], po[h][HD:HD + 1, :])
                    rrs.append(rr)
                for h in range(2):
                    rb = pool_rb.tile([HD, SQ], BF, tag="rb",
                                      name=f"rb{a}{j}{h}")
                    nc.gpsimd.partition_broadcast(rb[:], rrs[h][:])
                    rbs.append(rb)
                for h in range(2):
                    hb = h * HD
                    nc.vector.tensor_mul(
                        o_tiles[a][hb:hb + HD, j * SQ:(j + 1) * SQ],
                        po[h][0:HD, :],
                        rbs[h][:])
            return nee if nxt else None

        q1 = qproj_groups(1)
        q2 = qproj_groups(2)
        q3 = qproj_groups(3)
        k3, k3_dma = kproj_fillers(3)
        op01 = outproj_groups(0, (0, 1, 2, 3))
        op23_0 = outproj_groups(1, (0,))
        op23_1 = outproj_groups(1, (1,))
        op23_2t = outproj_groups(1, (2,), copy_eng="alt", pool_alt=True)

        plan = [
            (0, 0, vt_fill[2:] + [q0[1]], 3, ()),
            (0, 1, [q0[2], q1[0]], 1, ()),
            (0, 2, [q0[3], q1[1]], 1, ()),
            (0, 3, [q1[2], q1[3], lambda: k3_dma(0)], 1, ()),
            (1, 0, [k3[0], q2[0]], 1, ()),
            (1, 1, [k3[1], q2[1]], 1, ()),
            (1, 2, [k3[2], q2[2]], 1, ()),
            (1, 3, [k3[3], q2[3]], 1, ()),
            (2, 0, [q3[0]] + op01[0:4], 1, ()),
            (2, 1, [q3[1]] + op01[4:8], 1, ()),
            (2, 2, [q3[2]] + op01[8:12], 1, ()),
            (2, 3, [q3[3]] + op01[12:16], 1, ()),
            (3, 0, op01[16:24], 1, ()),
            (3, 1, op01[24:32], 1, ()),
            (3, 2, op23_0, 1, ()),
            # tail fillers (alt copies) hide the final norm chain
            (3, 3, op23_1, 1, op23_2t),
        ]
        pre = None
        for bi, (a, j, fillers, off, tails) in enumerate(plan):
            nxt = plan[bi + 1][0:2] if bi + 1 < len(plan) else None
            pre = attn_block(a, j, fillers, pre, nxt, pace_off=off,
                             tail_fillers=tails)
        for g in outproj_groups(1, (3,), copy_eng="alt", pool_alt=True):
            g()

    nc.compile()
    _CACHE["nc"] = nc
    return nc


def _tox(a):
    """[1024|512, N] -> [128, k, N] bf16 (partition-major k-tiling)."""
    r = a.shape[0] // 128
    return np.ascontiguousarray(
        a.reshape(r, 128, a.shape[1]).transpose(1, 0, 2)).astype(NPBF)


def kernel(Q, K, V, Wq, bq, Wk, bk, Wv, bv, Wo, bo):
    Q = np.asarray(Q, np.float32)
    K = np.asarray(K, np.float32)
    V = np.asarray(V, np.float32)
    Wq = np.asarray(Wq, np.float32)
    Wk = np.asarray(Wk, np.float32)
    Wv = np.asarray(Wv, np.float32)
    Wo = np.asarray(Wo, np.float32)
    bq = np.asarray(bq, np.float32)
    bk = np.asarray(bk, np.float32)
    bv = np.asarray(bv, np.float32)
    bo = np.asarray(bo, np.float32)
    scale = 1.0 / 8.0  # 1/sqrt(HD), folded into the q projection

    nc = _build_nc()
    in_maps = []
    for c in range(8):
        b, g = divmod(c, 2)
        gs = slice(g * GS, (g + 1) * GS)
        biasqk = np.empty((128, 8), np.float32)
        for m in range(NP):
            biasqk[:, m] = bq[gs][m * 128:(m + 1) * 128] * scale
            biasqk[:, 4 + m] = bk[gs][m * 128:(m + 1) * 128]
        in_maps.append({
            "xqT": _tox(Q[b].T),
            "xkT": _tox(K[b].T),
            "xvT": _tox(V[b].T),
            "wqT": _tox((Wq[gs] * scale).T),
            "wkT": _tox(Wk[gs].T),
            "wvT": _tox(Wv[gs].T),
            "woT": _tox(Wo[:, gs].T),
            "biasqk": biasqk,
        })

    host_bias = bo + Wo @ bv  # v bias folded through softmax + out-proj

    def run_and_gather():
        res = run_bass_kernel_spmd(nc, in_maps, list(range(8)))
        out = np.empty((B, S, D), np.float32)
        for b in range(B):
            acc = None
            for c in (2 * b, 2 * b + 1):
                for p in range(2):
                    part = np.asarray(res.results[c][f"outT{p}"])
                    part = part.transpose(1, 0, 2).reshape(D, S)
                    acc = part if acc is None else acc + part
            out[b] = acc.T + host_bias
        return out

    try:
        return run_and_gather()
    except Exception:
        # transient device wedge (e.g. NRT_EXEC_UNIT_UNRECOVERABLE) can
        # surface either in the run or in result materialization: retry once
        return run_and_gather()


# revision 21
# speedup vs baseline: 1.0090x; 1.0067x over previous
"""Multi-head attention (B=4, S=2048, D=1024, H=16) on 8 TRN2 NeuronCores.

Sharding: core c -> (batch b = c//2, head-group g = c%2): each core runs 8
heads of one batch (dout slice of 512) and emits two fp32 out-projection
partials (pairs 0-1 and 2-3); the host sums 4 partials per batch + bias.

All matmul operands are bf16 (fp32 PSUM accumulation); exp runs on the Act
engine (fp32 psum -> bf16); the softmax row-sum is folded into the AV matmul
via a ones-column on v; normalization = DVE reciprocal -> GPSIMD
partition_broadcast -> DVE multiply (no PE involvement). v-projection is
computed directly in transposed [seq, dout] layout (no PE transposes). The
v bias is folded into the host-side output bias (softmax rows sum to 1).

Schedule: k-proj (pairs 0-2) + q-proj(pair0, chunk0) prologue with
interleaved DMA sequencing; 16 attention blocks (pair a, 512-wide query
chunk j) run a depth-2 software pipeline over key-tile PAIRS (step p emits
fillers, QK/exp(p+2), AV(p)); scores for (2p, 2p+1) share one [128,2,512]
psum tile per head so exp stays 1024-wide. PSUM: score ring 2x[128,2,512]
(4 banks) + AV accumulators 2x[65,512] (2 banks) + a DEDICATED filler pool
2x[128,512] (2 banks) — filler matmuls (vT-proj, q/k-proj, out-proj) never
touch the score ring, which keeps its slot-reuse dependencies phase-locked
to one-period-old exps. The last two QK slots of each block prefetch the
next block; out-proj groups depending on the final normalization run as
tail fillers (alt-engine copies hide the norm chain) and a short epilogue.
PE is the critical engine at ~94% busy (~331 us of 354 us total); Act/exp
~272 us, DVE ~112 us, DMA ~105 us, Pool ~26 us all hide under it.
"""
from contextlib import ExitStack

import ml_dtypes
import numpy as np

import concourse.bacc as bacc
import concourse.tile as tile
from concourse import mybir
from concourse.bass_utils import run_bass_kernel_spmd

F32 = mybir.dt.float32
F16 = mybir.dt.float16
ALU_ADD = mybir.AluOpType.add
BF = mybir.dt.bfloat16
AF = mybir.ActivationFunctionType
NPBF = ml_dtypes.bfloat16

B, S, D, H, HD = 4, 2048, 1024, 16, 64
GS = D // 2            # 512: per-core dout slice (8 heads, 4 pairs)
NP = GS // 128         # 4 head pairs (= dout tiles = wo k-tiles)
NK = D // 128          # 8 din k-tiles
NSK = S // 128         # 16 key tiles
SQ = 512               # query chunk (block width)
NSQ = S // SQ          # 4
NIP = NSK // 2         # 8 key-tile pairs per block
NCH = S // 512         # 4 (512-wide chunks of S)

_CACHE = {}


def _build_nc():
    if "nc" in _CACHE:
        return _CACHE["nc"]

    nc = bacc.Bacc()

    xqT = nc.dram_tensor("xqT", [128, NK, S], BF, kind="ExternalInput")
    xkT = nc.dram_tensor("xkT", [128, NK, S], BF, kind="ExternalInput")
    xvT = nc.dram_tensor("xvT", [128, NK, S], BF, kind="ExternalInput")
    wqT = nc.dram_tensor("wqT", [128, NK, GS], BF, kind="ExternalInput")
    wkT = nc.dram_tensor("wkT", [128, NK, GS], BF, kind="ExternalInput")
    wvT = nc.dram_tensor("wvT", [128, NK, GS], BF, kind="ExternalInput")
    woT = nc.dram_tensor("woT", [128, NP, D], BF, kind="ExternalInput")
    biasqk = nc.dram_tensor("biasqk", [128, 8], F32, kind="ExternalInput")
    outTs = [nc.dram_tensor(f"outT{p}", [128, NK, S], F16,
                            kind="ExternalOutput") for p in range(2)]

    with tile.TileContext(nc) as tc, ExitStack() as kctx:
        consts = kctx.enter_context(tc.tile_pool(name="consts", bufs=1))
        pool_w = kctx.enter_context(tc.tile_pool(name="wp", bufs=1))
        pool_xq = kctx.enter_context(tc.tile_pool(name="xqp", bufs=1))
        pool_xs = kctx.enter_context(tc.tile_pool(name="xsp", bufs=3))
        pool_k = kctx.enter_context(tc.tile_pool(name="kTp", bufs=1))
        pool_q = kctx.enter_context(tc.tile_pool(name="qTp", bufs=4))
        pool_va = kctx.enter_context(tc.tile_pool(name="vap", bufs=1))
        pool_e = kctx.enter_context(tc.tile_pool(name="ep", bufs=8))
        pool_oT = kctx.enter_context(tc.tile_pool(name="oTp", bufs=1))
        pool_rr = kctx.enter_context(tc.tile_pool(name="rrp", bufs=3))
        pool_rb = kctx.enter_context(tc.tile_pool(name="rbp", bufs=3))
        pool_oo = kctx.enter_context(tc.tile_pool(name="oop", bufs=12))
        pp_qk = kctx.enter_context(tc.tile_pool(name="ppqk", bufs=2,
                                                space="PSUM"))
        pp_av = kctx.enter_context(tc.tile_pool(name="ppav", bufs=2,
                                                space="PSUM"))
        pp_fill = kctx.enter_context(tc.tile_pool(name="ppfl", bufs=2,
                                                  space="PSUM"))

        bias_t = consts.tile([128, 8], F32)

        # ---------------- static SBUF tensors ----------------
        wk_t = pool_w.tile([128, NK, GS], BF, name="wk")
        wq_t = pool_w.tile([128, NK, GS], BF, name="wq")
        wv_t = pool_w.tile([128, NK, GS], BF, name="wv")
        wo_t = pool_w.tile([128, NP, D], BF, name="wo")
        xq_t = pool_xq.tile([128, NK, S], BF, name="xq")
        kT = [pool_k.tile([128, S], BF, name=f"kT{m}") for m in range(NP)]
        v_aug = [pool_va.tile([128, 8, HD + 1], BF, name=f"va{i}")
                 for i in range(NSK)]
        o_tiles = [pool_oT.tile([128, S], BF, name=f"oT{a}")
                   for a in range(NP)]
        q_tiles = {}

        # ---------------- prologue: k-proj (all pairs) ----------------
        nc.sync.dma_start(out=wk_t[:, 0:4, 0:256], in_=wkT[:, 0:4, 0:256])
        xk_tiles = {}

        def xk_dma(n, split=False):
            xk_tiles[n] = pool_xs.tile([128, NK, 512], BF, tag="xs",
                                       name=f"xk{n}")
            if split:
                for q in range(4):
                    nc.sync.dma_start(
                        out=xk_tiles[n][:, 2 * q:2 * q + 2, :],
                        in_=xkT[:, 2 * q:2 * q + 2,
                                n * 512:(n + 1) * 512])
            else:
                nc.sync.dma_start(out=xk_tiles[n],
                                  in_=xkT[:, :, n * 512:(n + 1) * 512])

        def kproj_group(m, n, tiles):
            ps = pp_fill.tile([128, 512], F32, tag="fl", name=f"psk{n}{m}")
            for kk in range(NK):
                nc.tensor.matmul(
                    ps[:],
                    wk_t[:, kk, m * 128:(m + 1) * 128],
                    tiles[n][:, kk, :],
                    start=(kk == 0),
                    stop=(kk == NK - 1),
                )
            nc.vector.tensor_scalar_add(
                kT[m][:, n * 512:(n + 1) * 512], ps[:],
                bias_t[:, 4 + m:5 + m])

        # prologue covers pairs 0-2; pair 3 runs as mid-span fillers
        xk_dma(0, split=True)
        nc.sync.dma_start(out=wk_t[:, 4:8, 0:256], in_=wkT[:, 4:8, 0:256])
        nc.sync.dma_start(out=bias_t, in_=biasqk[:, :])
        nc.sync.dma_start(out=wk_t[:, :, 256:512], in_=wkT[:, :, 256:512])
        xk_dma(1)
        for n in range(NCH):
            if n + 2 < NCH:
                xk_dma(n + 2)
            if n == 2:
                nc.sync.dma_start(out=wq_t, in_=wqT[:, :, :])
            if n == 3:
                nc.sync.dma_start(out=wv_t, in_=wvT[:, :, :])
            for m in range(3):
                kproj_group(m, n, xk_tiles)

        def kproj_fillers(m):
            """4 filler groups for k-proj of pair m (re-streams xk)."""
            tiles = {}

            def dma(n):
                tiles[n] = pool_xs.tile([128, NK, 512], BF, tag="xs",
                                        name=f"xk{m}_{n}")
                nc.sync.dma_start(out=tiles[n],
                                  in_=xkT[:, :, n * 512:(n + 1) * 512])

            def group(n):
                def run():
                    if n + 1 < NCH:
                        dma(n + 1)
                    kproj_group(m, n, tiles)
                return run
            return [group(n) for n in range(NCH)], dma

        def xq_dma(n):
            nc.sync.dma_start(out=xq_t[:, :, n * 512:(n + 1) * 512],
                              in_=xqT[:, :, n * 512:(n + 1) * 512])

        def qproj_groups(a):
            qt = pool_q.tile([128, S], BF, tag="qT", name=f"qT{a}")
            q_tiles[a] = qt

            def group(n):
                def run():
                    ps = pp_fill.tile([128, 512], F32, tag="fl",
                                     name=f"psq{a}{n}")
                    for kk in range(NK):
                        nc.tensor.matmul(
                            ps[:],
                            wq_t[:, kk, a * 128:(a + 1) * 128],
                            xq_t[:, kk, n * 512:(n + 1) * 512],
                            start=(kk == 0),
                            stop=(kk == NK - 1),
                        )
                    nc.vector.tensor_scalar_add(
                        qt[:, n * 512:(n + 1) * 512], ps[:],
                        bias_t[:, a:a + 1])
                return run
            return [group(n) for n in range(NCH)]

        q0 = qproj_groups(0)

        # ---------------- vT-proj groups (one per seq-tile st) ----------
        xv_tiles = {}

        def xv_dma(n):
            xv_tiles[n] = pool_xs.tile([128, NK, 512], BF, tag="xs",
                                       name=f"xv{n}")
            nc.sync.dma_start(out=xv_tiles[n],
                              in_=xvT[:, :, n * 512:(n + 1) * 512])

        def vt_group(st):
            def run():
                n, sl = st // 4, st % 4
                if sl == 0 and 1 <= n < NCH - 1:
                    xv_dma(n + 1)
                ps = pp_fill.tile([128, 512], F32, tag="fl", name=f"psv{st}")
                for kk in range(NK):
                    nc.tensor.matmul(
                        ps[:],
                        xv_tiles[n][:, kk, sl * 128:(sl + 1) * 128],
                        wv_t[:, kk, :],
                        start=(kk == 0),
                        stop=(kk == NK - 1),
                    )
                nc.vector.memset(v_aug[st][:, :, HD:HD + 1], 1.0)
                nc.vector.tensor_copy(v_aug[st][:, :, 0:HD], ps[:])
            return run

        vt_fill = [vt_group(st) for st in range(NSK)]
        xq_dma(0)
        xv_dma(0)
        q0[0]()
        xq_dma(1)
        vt_fill[0]()
        xv_dma(1)
        vt_fill[1]()
        xq_dma(2)
        xq_dma(3)

        nc.sync.dma_start(out=wo_t, in_=woT[:, :, :])

        # ---------------- out-proj groups ----------------
        def outproj_groups(p, jjs, copy_eng="vector", pool_alt=False):
            def group(dm, jj, gi):
                def run():
                    if pool_alt and gi % 2:
                        # post-exp groups: the score ring is idle, borrow its
                        # slots to double the effective psum ring depth
                        ps = pp_qk.tile([128, 512], F32, tag="sc",
                                        name=f"pso{p}{dm}{jj}")
                    else:
                        ps = pp_fill.tile([128, 512], F32, tag="fl",
                                         name=f"pso{p}{dm}{jj}")
                    for a in (2 * p, 2 * p + 1):
                        nc.tensor.matmul(
                            ps[:],
                            wo_t[:, a, dm * 128:(dm + 1) * 128],
                            o_tiles[a][:, jj * 512:(jj + 1) * 512],
                            start=(a == 2 * p),
                            stop=(a == 2 * p + 1),
                        )
                    oo = pool_oo.tile([128, 512], F16, tag="oo",
                                      name=f"oo{p}{dm}{jj}")
                    use_act = (copy_eng == "scalar"
                               or (copy_eng == "alt" and gi % 2))
                    with nc.allow_low_precision(reason="fp16 partial out"):
                        if use_act:
                            nc.scalar.copy(oo[:], ps[:])
                        else:
                            nc.vector.tensor_copy(oo[:], ps[:])
                    nc.sync.dma_start(
                        out=outTs[p][:, dm, jj * 512:(jj + 1) * 512],
                        in_=oo[:])
                return run
            return [group(dm, jj, gi)
                    for gi, (jj, dm) in enumerate(
                        (jj, dm) for jj in jjs for dm in range(NK))]

        # ---------------- attention ----------------
        def make_qk_exp(a, j, ee):
            def qk_exp(p):
                for h in range(2):
                    hb = h * HD
                    sc = pp_qk.tile([128, 2, 512], F32, tag="sc",
                                    name=f"sc{a}{j}{p}{h}")
                    for u in range(2):
                        i = 2 * p + u
                        nc.tensor.matmul(
                            sc[:, u, :],
                            kT[a][hb:hb + HD, i * 128:(i + 1) * 128],
                            q_tiles[a][hb:hb + HD,
                                       j * SQ:(j + 1) * SQ],
                            start=True,
                            stop=True,
                        )
                    e = pool_e.tile([128, 2, 512], BF, tag="e",
                                    name=f"e{a}{j}{p}{h}")
                    nc.scalar.activation(e[:], sc[:], AF.Exp)
                    ee[(p, h)] = e
            return qk_exp

        def attn_block(a, j, fillers, pre, nxt, pace_off=1,
                       tail_fillers=()):
            """Depth-2 software pipeline over key-tile PAIRS: step p emits
            [fillers, QK/exp(p+2), AV(p)]. Scores for (2p, 2p+1) share one
            [128,2,512] psum tile per head so exp stays at 1024-wide; filler
            matmuls use their own pp_fill pool and never touch the score
            ring. The last two QK slots prefetch the next block."""
            po = [pp_av.tile([HD + 1, SQ], F32, tag="po",
                             name=f"po{a}{j}{h}") for h in range(2)]
            ee = pre if pre is not None else {}
            qk_exp = make_qk_exp(a, j, ee)
            if pre is None:
                qk_exp(0)
                qk_exp(1)
            nee = {}
            nqk = make_qk_exp(nxt[0], nxt[1], nee) if nxt else None
            nf, fi = len(fillers), 0
            for p in range(NIP):
                want = (p + pace_off) * nf // NIP
                while fi < min(want, nf):
                    fillers[fi]()
                    fi += 1
                if p + 2 < NIP:
                    qk_exp(p + 2)
                elif nqk is not None:
                    nqk(p + 2 - NIP)
                for h in range(2):
                    e = ee.pop((p, h))
                    for u in range(2):
                        i = 2 * p + u
                        nc.tensor.matmul(
                            po[h][:],
                            v_aug[i][:, 2 * a + h, :],
                            e[:, u, :],
                            start=(i == 0),
                            stop=(i == NSK - 1),
                        )
            while fi < nf:
                fillers[fi]()
                fi += 1
            for g in tail_fillers:
                g()
            # normalization: 1/rowsum broadcast (Pool) and multiply (DVE);
            # recips first so the h0/h1 chains overlap across engines
            with nc.allow_low_precision(reason="bf16 softmax reciprocal"):
                rrs, rbs = [], []
                for h in range(2):
                    rr = pool_rr.tile([1, SQ], BF, tag="rr",
                                      name=f"rr{a}{j}{h}")
                    nc.vector.reciprocal(rr[:], po[h][HD:HD + 1, :])
                    rrs.append(rr)
                for h in range(2):
                    rb = pool_rb.tile([HD, SQ], BF, tag="rb",
                                      name=f"rb{a}{j}{h}")
                    nc.gpsimd.partition_broadcast(rb[:], rrs[h][:])
                    rbs.append(rb)
                for h in range(2):
                    hb = h * HD
                    nc.vector.tensor_mul(
                        o_tiles[a][hb:hb + HD, j * SQ:(j + 1) * SQ],
                        po[h][0:HD, :],
                        rbs[h][:])
            return nee if nxt else None

        q1 = qproj_groups(1)
        q2 = qproj_groups(2)
        q3 = qproj_groups(3)
        k3, k3_dma = kproj_fillers(3)
        op01 = outproj_groups(0, (0, 1, 2, 3))
        op23_0 = outproj_groups(1, (0,))
        op23_1 = outproj_groups(1, (1,))
        op23_2t = outproj_groups(1, (2,), copy_eng="alt", pool_alt=True)

        plan = [
            (0, 0, vt_fill[2:] + [q0[1]], 3, ()),
            (0, 1, [q0[2], q1[0]], 1, ()),
            (0, 2, [q0[3], q1[1]], 1, ()),
            (0, 3, [q1[2], q1[3], lambda: k3_dma(0)], 1, ()),
            (1, 0, [k3[0], q2[0]], 1, ()),
            (1, 1, [k3[1], q2[1]], 1, ()),
            (1, 2, [k3[2], q2[2]], 1, ()),
            (1, 3, [k3[3], q2[3]], 1, ()),
            (2, 0, [q3[0]] + op01[0:4], 1, ()),
            (2, 1, [q3[1]] + op01[4:8], 1, ()),
            (2, 2, [q3[2]] + op01[8:12], 1, ()),
            (2, 3, [q3[3]] + op01[12:16], 1, ()),
            (3, 0, op01[16:24], 1, ()),
            (3, 1, op01[24:32], 1, ()),
            (3, 2, op23_0, 1, ()),
            # tail fillers (alt copies) hide the final norm chain
            (3, 3, op23_1, 1, op23_2t),
        ]
        pre = None
        for bi, (a, j, fillers, off, tails) in enumerate(plan):
            nxt = plan[bi + 1][0:2] if bi + 1 < len(plan) else None
            pre = attn_block(a, j, fillers, pre, nxt, pace_off=off,
                             tail_fillers=tails)
        for g in outproj_groups(1, (3,), copy_eng="alt", pool_alt=True):
            g()

    nc.compile()
    _CACHE["nc"] = nc
    return nc


def _tox(a):
    """[1024|512, N] -> [128, k, N] bf16 (partition-major k-tiling)."""
    r = a.shape[0] // 128
    return np.ascontiguousarray(
        a.reshape(r, 128, a.shape[1]).transpose(1, 0, 2)).astype(NPBF)


def kernel(Q, K, V, Wq, bq, Wk, bk, Wv, bv, Wo, bo):
    Q = np.asarray(Q, np.float32)
    K = np.asarray(K, np.float32)
    V = np.asarray(V, np.float32)
    Wq = np.asarray(Wq, np.float32)
    Wk = np.asarray(Wk, np.float32)
    Wv = np.asarray(Wv, np.float32)
    Wo = np.asarray(Wo, np.float32)
    bq = np.asarray(bq, np.float32)
    bk = np.asarray(bk, np.float32)
    bv = np.asarray(bv, np.float32)
    bo = np.asarray(bo, np.float32)
    scale = 1.0 / 8.0  # 1/sqrt(HD), folded into the q projection

    nc = _build_nc()
    in_maps = []
    for c in range(8):
        b, g = divmod(c, 2)
        gs = slice(g * GS, (g + 1) * GS)
        biasqk = np.empty((128, 8), np.float32)
        for m in range(NP):
            biasqk[:, m] = bq[gs][m * 128:(m + 1) * 128] * scale
            biasqk[:, 4 + m] = bk[gs][m * 128:(m + 1) * 128]
        in_maps.append({
            "xqT": _tox(Q[b].T),
            "xkT": _tox(K[b].T),
            "xvT": _tox(V[b].T),
            "wqT": _tox((Wq[gs] * scale).T),
            "wkT": _tox(Wk[gs].T),
            "wvT": _tox(Wv[gs].T),
            "woT": _tox(Wo[:, gs].T),
            "biasqk": biasqk,
        })

    host_bias = bo + Wo @ bv  # v bias folded through softmax + out-proj

    def run_and_gather():
        res = run_bass_kernel_spmd(nc, in_maps, list(range(8)))
        out = np.empty((B, S, D), np.float32)
        for b in range(B):
            acc = None
            for c in (2 * b, 2 * b + 1):
                for p in range(2):
                    part = np.asarray(res.results[c][f"outT{p}"])
                    part = part.transpose(1, 0, 2).reshape(D, S)
                    acc = part if acc is None else acc + part
            out[b] = acc.T + host_bias
        return out

    try:
        return run_and_gather()
    except Exception:
        # transient device wedge (e.g. NRT_EXEC_UNIT_UNRECOVERABLE) can
        # surface either in the run or in result materialization: retry once
        return run_and_gather()


# revision 23
# speedup vs baseline: 1.0097x; 1.0008x over previous
"""Multi-head attention (B=4, S=2048, D=1024, H=16) on 8 TRN2 NeuronCores.

Sharding: core c -> (batch b = c//2, head-group g = c%2): each core runs 8
heads of one batch (dout slice of 512) and emits two fp16 out-projection
partials (pairs 0-1 and 2-3); the host sums 4 partials per batch + bias.

All matmul operands are bf16 (fp32 PSUM accumulation); exp runs on the Act
engine (fp32 psum -> bf16); the softmax row-sum is folded into the AV matmul
via a ones-column on v; normalization = DVE reciprocal -> GPSIMD
partition_broadcast -> DVE multiply (no PE involvement). v-projection is
computed directly in transposed [seq, dout] layout (no PE transposes). The
v bias is folded into the host-side output bias (softmax rows sum to 1).

Schedule: k-proj (pairs 0-2) + q-proj(pair0, chunk0) prologue with
interleaved DMA sequencing; 16 attention blocks (pair a, 512-wide query
chunk j) run a depth-2 software pipeline over key-tile PAIRS (step p emits
fillers, QK/exp(p+2), AV(p)); scores for (2p, 2p+1) share one [128,2,512]
psum tile per head so exp stays 1024-wide. PSUM: score ring 2x[128,2,512]
(4 banks) + AV accumulators 2x[65,512] (2 banks) + a DEDICATED filler pool
2x[128,512] (2 banks) — filler matmuls (vT-proj, q/k-proj, out-proj) never
touch the score ring, which keeps its slot-reuse dependencies phase-locked
to one-period-old exps. The last two QK slots of each block prefetch the
next block; out-proj groups depending on the final normalization run as
tail fillers (alt-engine copies hide the norm chain) and a short epilogue.
Out-projection partials are written as fp16 (halves the output DMA). PE is
the critical engine at ~94.5% busy (~331 us of 351 us total); Act/exp
~272 us, DVE ~112 us, DMA ~84 us, Pool ~26 us all hide under it.
"""
from contextlib import ExitStack

import ml_dtypes
import numpy as np

import concourse.bacc as bacc
import concourse.tile as tile
from concourse import mybir
from concourse.bass_utils import run_bass_kernel_spmd

F32 = mybir.dt.float32
F16 = mybir.dt.float16
ALU_ADD = mybir.AluOpType.add
BF = mybir.dt.bfloat16
AF = mybir.ActivationFunctionType
NPBF = ml_dtypes.bfloat16

B, S, D, H, HD = 4, 2048, 1024, 16, 64
GS = D // 2            # 512: per-core dout slice (8 heads, 4 pairs)
NP = GS // 128         # 4 head pairs (= dout tiles = wo k-tiles)
NK = D // 128          # 8 din k-tiles
NSK = S // 128         # 16 key tiles
SQ = 512               # query chunk (block width)
NSQ = S // SQ          # 4
NIP = NSK // 2         # 8 key-tile pairs per block
NCH = S // 512         # 4 (512-wide chunks of S)

_CACHE = {}


def _build_nc():
    if "nc" in _CACHE:
        return _CACHE["nc"]

    nc = bacc.Bacc()

    xqT = nc.dram_tensor("xqT", [128, NK, S], BF, kind="ExternalInput")
    xkT = nc.dram_tensor("xkT", [128, NK, S], BF, kind="ExternalInput")
    xvT = nc.dram_tensor("xvT", [128, NK, S], BF, kind="ExternalInput")
    wqT = nc.dram_tensor("wqT", [128, NK, GS], BF, kind="ExternalInput")
    wkT = nc.dram_tensor("wkT", [128, NK, GS], BF, kind="ExternalInput")
    wvT = nc.dram_tensor("wvT", [128, NK, GS], BF, kind="ExternalInput")
    woT = nc.dram_tensor("woT", [128, NP, D], BF, kind="ExternalInput")
    biasqk = nc.dram_tensor("biasqk", [128, 8], F32, kind="ExternalInput")
    outTs = [nc.dram_tensor(f"outT{p}", [128, NK, S], F16,
                            kind="ExternalOutput") for p in range(2)]

    with tile.TileContext(nc) as tc, ExitStack() as kctx:
        consts = kctx.enter_context(tc.tile_pool(name="consts", bufs=1))
        pool_w = kctx.enter_context(tc.tile_pool(name="wp", bufs=1))
        pool_xq = kctx.enter_context(tc.tile_pool(name="xqp", bufs=1))
        pool_xs = kctx.enter_context(tc.tile_pool(name="xsp", bufs=3))
        pool_k = kctx.enter_context(tc.tile_pool(name="kTp", bufs=1))
        pool_q = kctx.enter_context(tc.tile_pool(name="qTp", bufs=4))
        pool_va = kctx.enter_context(tc.tile_pool(name="vap", bufs=1))
        pool_e = kctx.enter_context(tc.tile_pool(name="ep", bufs=8))
        pool_oT = kctx.enter_context(tc.tile_pool(name="oTp", bufs=1))
        pool_rr = kctx.enter_context(tc.tile_pool(name="rrp", bufs=3))
        pool_rb = kctx.enter_context(tc.tile_pool(name="rbp", bufs=3))
        pool_oo = kctx.enter_context(tc.tile_pool(name="oop", bufs=12))
        pool_ob = kctx.enter_context(tc.tile_pool(name="obp", bufs=1))
        pp_qk = kctx.enter_context(tc.tile_pool(name="ppqk", bufs=2,
                                                space="PSUM"))
        pp_av = kctx.enter_context(tc.tile_pool(name="ppav", bufs=2,
                                                space="PSUM"))
        pp_fill = kctx.enter_context(tc.tile_pool(name="ppfl", bufs=2,
                                                  space="PSUM"))

        bias_t = consts.tile([128, 8], F32)

        # ---------------- static SBUF tensors ----------------
        wk_t = pool_w.tile([128, NK, GS], BF, name="wk")
        wq_t = pool_w.tile([128, NK, GS], BF, name="wq")
        wv_t = pool_w.tile([128, NK, GS], BF, name="wv")
        wo_t = pool_w.tile([128, NP, D], BF, name="wo")
        xq_t = pool_xq.tile([128, NK, S], BF, name="xq")
        kT = [pool_k.tile([128, S], BF, name=f"kT{m}") for m in range(NP)]
        v_aug = [pool_va.tile([128, 8, HD + 1], BF, name=f"va{i}")
                 for i in range(NSK)]
        o_tiles = [pool_oT.tile([128, S], BF, name=f"oT{a}")
                   for a in range(NP)]
        q_tiles = {}

        # ---------------- prologue: k-proj (all pairs) ----------------
        nc.sync.dma_start(out=wk_t[:, 0:4, 0:256], in_=wkT[:, 0:4, 0:256])
        xk_tiles = {}

        def xk_dma(n, split=False):
            xk_tiles[n] = pool_xs.tile([128, NK, 512], BF, tag="xs",
                                       name=f"xk{n}")
            if split:
                for q in range(4):
                    nc.sync.dma_start(
                        out=xk_tiles[n][:, 2 * q:2 * q + 2, :],
                        in_=xkT[:, 2 * q:2 * q + 2,
                                n * 512:(n + 1) * 512])
            else:
                nc.sync.dma_start(out=xk_tiles[n],
                                  in_=xkT[:, :, n * 512:(n + 1) * 512])

        def kproj_group(m, n, tiles):
            ps = pp_fill.tile([128, 512], F32, tag="fl", name=f"psk{n}{m}")
            for kk in range(NK):
                nc.tensor.matmul(
                    ps[:],
                    wk_t[:, kk, m * 128:(m + 1) * 128],
                    tiles[n][:, kk, :],
                    start=(kk == 0),
                    stop=(kk == NK - 1),
                )
            nc.vector.tensor_scalar_add(
                kT[m][:, n * 512:(n + 1) * 512], ps[:],
                bias_t[:, 4 + m:5 + m])

        # prologue covers pairs 0-2; pair 3 runs as mid-span fillers
        xk_dma(0, split=True)
        nc.sync.dma_start(out=wk_t[:, 4:8, 0:256], in_=wkT[:, 4:8, 0:256])
        nc.sync.dma_start(out=bias_t, in_=biasqk[:, :])
        nc.sync.dma_start(out=wk_t[:, :, 256:512], in_=wkT[:, :, 256:512])
        xk_dma(1)
        for n in range(NCH):
            if n + 2 < NCH:
                xk_dma(n + 2)
            if n == 2:
                nc.sync.dma_start(out=wq_t, in_=wqT[:, :, :])
            if n == 3:
                nc.sync.dma_start(out=wv_t, in_=wvT[:, :, :])
            for m in range(3):
                kproj_group(m, n, xk_tiles)

        def kproj_fillers(m):
            """4 filler groups for k-proj of pair m (re-streams xk)."""
            tiles = {}

            def dma(n):
                tiles[n] = pool_xs.tile([128, NK, 512], BF, tag="xs",
                                        name=f"xk{m}_{n}")
                nc.sync.dma_start(out=tiles[n],
                                  in_=xkT[:, :, n * 512:(n + 1) * 512])

            def group(n):
                def run():
                    if n + 1 < NCH:
                        dma(n + 1)
                    kproj_group(m, n, tiles)
                return run
            return [group(n) for n in range(NCH)], dma

        def xq_dma(n):
            nc.sync.dma_start(out=xq_t[:, :, n * 512:(n + 1) * 512],
                              in_=xqT[:, :, n * 512:(n + 1) * 512])

        def qproj_groups(a):
            qt = pool_q.tile([128, S], BF, tag="qT", name=f"qT{a}")
            q_tiles[a] = qt

            def group(n):
                def run():
                    ps = pp_fill.tile([128, 512], F32, tag="fl",
                                     name=f"psq{a}{n}")
                    for kk in range(NK):
                        nc.tensor.matmul(
                            ps[:],
                            wq_t[:, kk, a * 128:(a + 1) * 128],
                            xq_t[:, kk, n * 512:(n + 1) * 512],
                            start=(kk == 0),
                            stop=(kk == NK - 1),
                        )
                    nc.vector.tensor_scalar_add(
                        qt[:, n * 512:(n + 1) * 512], ps[:],
                        bias_t[:, a:a + 1])
                return run
            return [group(n) for n in range(NCH)]

        q0 = qproj_groups(0)

        # ---------------- vT-proj groups (one per seq-tile st) ----------
        xv_tiles = {}

        def xv_dma(n):
            xv_tiles[n] = pool_xs.tile([128, NK, 512], BF, tag="xs",
                                       name=f"xv{n}")
            nc.sync.dma_start(out=xv_tiles[n],
                              in_=xvT[:, :, n * 512:(n + 1) * 512])

        def vt_group(st):
            def run():
                n, sl = st // 4, st % 4
                if sl == 0 and 1 <= n < NCH - 1:
                    xv_dma(n + 1)
                ps = pp_fill.tile([128, 512], F32, tag="fl", name=f"psv{st}")
                for kk in range(NK):
                    nc.tensor.matmul(
                        ps[:],
                        xv_tiles[n][:, kk, sl * 128:(sl + 1) * 128],
                        wv_t[:, kk, :],
                        start=(kk == 0),
                        stop=(kk == NK - 1),
                    )
                nc.vector.memset(v_aug[st][:, :, HD:HD + 1], 1.0)
                nc.vector.tensor_copy(v_aug[st][:, :, 0:HD], ps[:])
            return run

        vt_fill = [vt_group(st) for st in range(NSK)]
        xq_dma(0)
        xv_dma(0)
        q0[0]()
        xq_dma(1)
        vt_fill[0]()
        xv_dma(1)
        vt_fill[1]()
        xq_dma(2)
        xq_dma(3)

        nc.sync.dma_start(out=wo_t, in_=woT[:, :, :])

        # ---------------- out-proj groups ----------------
        def outproj_groups(p, jjs, copy_eng="vector", pool_alt=False):
            def group(dm, jj, gi):
                def run():
                    if pool_alt and gi % 2:
                        # post-exp groups: the score ring is idle, borrow its
                        # slots to double the effective psum ring depth
                        ps = pp_qk.tile([128, 512], F32, tag="sc",
                                        name=f"pso{p}{dm}{jj}")
                    else:
                        ps = pp_fill.tile([128, 512], F32, tag="fl",
                                         name=f"pso{p}{dm}{jj}")
                    for a in (2 * p, 2 * p + 1):
                        nc.tensor.matmul(
                            ps[:],
                            wo_t[:, a, dm * 128:(dm + 1) * 128],
                            o_tiles[a][:, jj * 512:(jj + 1) * 512],
                            start=(a == 2 * p),
                            stop=(a == 2 * p + 1),
                        )
                    oo = pool_oo.tile([128, 512], F16, tag="oo",
                                      name=f"oo{p}{dm}{jj}")
                    use_act = (copy_eng == "scalar"
                               or (copy_eng == "alt" and gi % 2))
                    with nc.allow_low_precision(reason="fp16 partial out"):
                        if use_act:
                            nc.scalar.copy(oo[:], ps[:])
                        else:
                            nc.vector.tensor_copy(oo[:], ps[:])
                    nc.sync.dma_start(
                        out=outTs[p][:, dm, jj * 512:(jj + 1) * 512],
                        in_=oo[:])
                return run
            return [group(dm, jj, gi)
                    for gi, (jj, dm) in enumerate(
                        (jj, dm) for jj in jjs for dm in range(NK))]

        # ---------------- attention ----------------
        def make_qk_exp(a, j, ee):
            def qk_exp(p):
                for h in range(2):
                    hb = h * HD
                    sc = pp_qk.tile([128, 2, 512], F32, tag="sc",
                                    name=f"sc{a}{j}{p}{h}")
                    for u in range(2):
                        i = 2 * p + u
                        nc.tensor.matmul(
                            sc[:, u, :],
                            kT[a][hb:hb + HD, i * 128:(i + 1) * 128],
                            q_tiles[a][hb:hb + HD,
                                       j * SQ:(j + 1) * SQ],
                            start=True,
                            stop=True,
                        )
                    e = pool_e.tile([128, 2, 512], BF, tag="e",
                                    name=f"e{a}{j}{p}{h}")
                    nc.scalar.activation(e[:], sc[:], AF.Exp)
                    ee[(p, h)] = e
            return qk_exp

        def attn_block(a, j, fillers, pre, nxt, pace_off=1,
                       tail_fillers=()):
            """Depth-2 software pipeline over key-tile PAIRS: step p emits
            [fillers, QK/exp(p+2), AV(p)]. Scores for (2p, 2p+1) share one
            [128,2,512] psum tile per head so exp stays at 1024-wide; filler
            matmuls use their own pp_fill pool and never touch the score
            ring. The last two QK slots prefetch the next block."""
            po = [pp_av.tile([HD + 1, SQ], F32, tag="po",
                             name=f"po{a}{j}{h}") for h in range(2)]
            ee = pre if pre is not None else {}
            qk_exp = make_qk_exp(a, j, ee)
            if pre is None:
                qk_exp(0)
                qk_exp(1)
            nee = {}
            nqk = make_qk_exp(nxt[0], nxt[1], nee) if nxt else None
            nf, fi = len(fillers), 0
            for p in range(NIP):
                want = (p + pace_off) * nf // NIP
                while fi < min(want, nf):
                    fillers[fi]()
                    fi += 1
                if p + 2 < NIP:
                    qk_exp(p + 2)
                elif nqk is not None:
                    nqk(p + 2 - NIP)
                for h in range(2):
                    e = ee.pop((p, h))
                    for u in range(2):
                        i = 2 * p + u
                        nc.tensor.matmul(
                            po[h][:],
                            v_aug[i][:, 2 * a + h, :],
                            e[:, u, :],
                            start=(i == 0),
                            stop=(i == NSK - 1),
                        )
            while fi < nf:
                fillers[fi]()
                fi += 1
            for g in tail_fillers:
                g()
            # normalization: 1/rowsum broadcast (Pool) and multiply (DVE);
            # recips first so the h0/h1 chains overlap across engines
            with nc.allow_low_precision(reason="bf16 softmax reciprocal"):
                rrs, rbs = [], []
                for h in range(2):
                    rr = pool_rr.tile([1, SQ], BF, tag="rr",
                                      name=f"rr{a}{j}{h}")
                    nc.vector.reciprocal(rr[:], po[h][HD:HD + 1, :])
                    rrs.append(rr)
                for h in range(2):
                    rb = pool_rb.tile([HD, SQ], BF, tag="rb",
                                      name=f"rb{a}{j}{h}")
                    nc.gpsimd.partition_broadcast(rb[:], rrs[h][:])
                    rbs.append(rb)
                for h in range(2):
                    hb = h * HD
                    nc.vector.tensor_mul(
                        o_tiles[a][hb:hb + HD, j * SQ:(j + 1) * SQ],
                        po[h][0:HD, :],
                        rbs[h][:])
            return nee if nxt else None

        q1 = qproj_groups(1)
        q2 = qproj_groups(2)
        q3 = qproj_groups(3)
        k3, k3_dma = kproj_fillers(3)
        op01 = outproj_groups(0, (0, 1, 2, 3))
        op23_0 = outproj_groups(1, (0,))
        op23_1 = outproj_groups(1, (1,))
        op23_2t = outproj_groups(1, (2,), copy_eng="alt", pool_alt=True)

        plan = [
            (0, 0, vt_fill[2:] + [q0[1]], 3, ()),
            (0, 1, [q0[2], q1[0]], 1, ()),
            (0, 2, [q0[3], q1[1]], 1, ()),
            (0, 3, [q1[2], q1[3], lambda: k3_dma(0)], 1, ()),
            (1, 0, [k3[0], q2[0]], 1, ()),
            (1, 1, [k3[1], q2[1]], 1, ()),
            (1, 2, [k3[2], q2[2]], 1, ()),
            (1, 3, [k3[3], q2[3]], 1, ()),
            (2, 0, [q3[0]] + op01[0:4], 1, ()),
            (2, 1, [q3[1]] + op01[4:8], 1, ()),
            (2, 2, [q3[2]] + op01[8:12], 1, ()),
            (2, 3, [q3[3]] + op01[12:16], 1, ()),
            (3, 0, op01[16:24], 1, ()),
            (3, 1, op01[24:32], 1, ()),
            (3, 2, op23_0, 1, ()),
            # tail fillers (alt copies) hide the final norm chain
            (3, 3, op23_1, 1, op23_2t),
        ]
        pre = None
        for bi, (a, j, fillers, off, tails) in enumerate(plan):
            nxt = plan[bi + 1][0:2] if bi + 1 < len(plan) else None
            pre = attn_block(a, j, fillers, pre, nxt, pace_off=off,
                             tail_fillers=tails)
        # epilogue (out-proj pairs 2-3, jj=3): merge the 8 per-dm stores
        # into one staging tile and a single DMA so the final drain pays one
        # HWDGE/descriptor fixed cost instead of eight
        oo_big = pool_ob.tile([128, NK, 512], F16, name="oob")
        for dm in range(NK):
            if dm % 2:
                ps = pp_qk.tile([128, 512], F32, tag="sc",
                                name=f"pse{dm}")
            else:
                ps = pp_fill.tile([128, 512], F32, tag="fl",
                                 name=f"pse{dm}")
            for a in (2, 3):
                nc.tensor.matmul(
                    ps[:],
                    wo_t[:, a, dm * 128:(dm + 1) * 128],
                    o_tiles[a][:, 3 * 512:4 * 512],
                    start=(a == 2),
                    stop=(a == 3),
                )
            with nc.allow_low_precision(reason="fp16 partial out"):
                if dm % 2:
                    nc.scalar.copy(oo_big[:, dm, :], ps[:])
                else:
                    nc.vector.tensor_copy(oo_big[:, dm, :], ps[:])
        nc.sync.dma_start(out=outTs[1][:, :, 3 * 512:4 * 512],
                          in_=oo_big[:])

    nc.compile()
    _CACHE["nc"] = nc
    return nc


def _tox(a):
    """[1024|512, N] -> [128, k, N] bf16 (partition-major k-tiling)."""
    r = a.shape[0] // 128
    return np.ascontiguousarray(
        a.reshape(r, 128, a.shape[1]).transpose(1, 0, 2)).astype(NPBF)


def kernel(Q, K, V, Wq, bq, Wk, bk, Wv, bv, Wo, bo):
    Q = np.asarray(Q, np.float32)
    K = np.asarray(K, np.float32)
    V = np.asarray(V, np.float32)
    Wq = np.asarray(Wq, np.float32)
    Wk = np.asarray(Wk, np.float32)
    Wv = np.asarray(Wv, np.float32)
    Wo = np.asarray(Wo, np.float32)
    bq = np.asarray(bq, np.float32)
    bk = np.asarray(bk, np.float32)
    bv = np.asarray(bv, np.float32)
    bo = np.asarray(bo, np.float32)
    scale = 1.0 / 8.0  # 1/sqrt(HD), folded into the q projection

    nc = _build_nc()
    in_maps = []
    for c in range(8):
        b, g = divmod(c, 2)
        gs = slice(g * GS, (g + 1) * GS)
        biasqk = np.empty((128, 8), np.float32)
        for m in range(NP):
            biasqk[:, m] = bq[gs][m * 128:(m + 1) * 128] * scale
            biasqk[:, 4 + m] = bk[gs][m * 128:(m + 1) * 128]
        in_maps.append({
            "xqT": _tox(Q[b].T),
            "xkT": _tox(K[b].T),
            "xvT": _tox(V[b].T),
            "wqT": _tox((Wq[gs] * scale).T),
            "wkT": _tox(Wk[gs].T),
            "wvT": _tox(Wv[gs].T),
            "woT": _tox(Wo[:, gs].T),
            "biasqk": biasqk,
        })

    host_bias = bo + Wo @ bv  # v bias folded through softmax + out-proj

    def run_and_gather():
        res = run_bass_kernel_spmd(nc, in_maps, list(range(8)))
        out = np.empty((B, S, D), np.float32)
        for b in range(B):
            acc = None
            for c in (2 * b, 2 * b + 1):
                for p in range(2):
                    part = np.asarray(res.results[c][f"outT{p}"])
                    part = part.transpose(1, 0, 2).reshape(D, S)
                    acc = part if acc is None else acc + part
            out[b] = acc.T + host_bias
        return out

    try:
        return run_and_gather()
    except Exception:
        # transient device wedge (e.g. NRT_EXEC_UNIT_UNRECOVERABLE) can
        # surface either in the run or in result materialization: retry once
        return run_and_gather()


# revision 24
# speedup vs baseline: 1.0099x; 1.0001x over previous
"""Multi-head attention (B=4, S=2048, D=1024, H=16) on 8 TRN2 NeuronCores.

Sharding: core c -> (batch b = c//2, head-group g = c%2): each core runs 8
heads of one batch (dout slice of 512) and emits two fp16 out-projection
partials (pairs 0-1 and 2-3); the host sums 4 partials per batch + bias.

All matmul operands are bf16 (fp32 PSUM accumulation); exp runs on the Act
engine (fp32 psum -> bf16); the softmax row-sum is folded into the AV matmul
via a ones-column on v; normalization = DVE reciprocal -> GPSIMD
partition_broadcast -> DVE multiply (no PE involvement). v-projection is
computed directly in transposed [seq, dout] layout (no PE transposes). The
v bias is folded into the host-side output bias (softmax rows sum to 1).

Schedule: k-proj (pairs 0-2) + q-proj(pair0, chunk0) prologue with
interleaved DMA sequencing; 16 attention blocks (pair a, 512-wide query
chunk j) run a depth-2 software pipeline over key-tile PAIRS (step p emits
fillers, QK/exp(p+2), AV(p)); scores for (2p, 2p+1) share one [128,2,512]
psum tile per head so exp stays 1024-wide. PSUM: score ring 2x[128,2,512]
(4 banks) + AV accumulators 2x[65,512] (2 banks) + a DEDICATED filler pool
2x[128,512] (2 banks) — filler matmuls (vT-proj, q/k-proj, out-proj) never
touch the score ring, which keeps its slot-reuse dependencies phase-locked
to one-period-old exps. The last two QK slots of each block prefetch the
next block; out-proj groups depending on the final normalization run as
tail fillers (alt-engine copies hide the norm chain) and a short epilogue.
Out-projection partials are written as fp16 (halves the output DMA). PE is
the critical engine at ~94.5% busy (~331 us of 351 us total); Act/exp
~272 us, DVE ~112 us, DMA ~84 us, Pool ~26 us all hide under it.
"""
from contextlib import ExitStack

import ml_dtypes
import numpy as np

import concourse.bacc as bacc
import concourse.tile as tile
from concourse import mybir
from concourse.bass_utils import run_bass_kernel_spmd

F32 = mybir.dt.float32
F16 = mybir.dt.float16
ALU_ADD = mybir.AluOpType.add
BF = mybir.dt.bfloat16
AF = mybir.ActivationFunctionType
NPBF = ml_dtypes.bfloat16

B, S, D, H, HD = 4, 2048, 1024, 16, 64
GS = D // 2            # 512: per-core dout slice (8 heads, 4 pairs)
NP = GS // 128         # 4 head pairs (= dout tiles = wo k-tiles)
NK = D // 128          # 8 din k-tiles
NSK = S // 128         # 16 key tiles
SQ = 512               # query chunk (block width)
NSQ = S // SQ          # 4
NIP = NSK // 2         # 8 key-tile pairs per block
NCH = S // 512         # 4 (512-wide chunks of S)

_CACHE = {}


def _build_nc():
    if "nc" in _CACHE:
        return _CACHE["nc"]

    nc = bacc.Bacc()

    xqT = nc.dram_tensor("xqT", [128, NK, S], BF, kind="ExternalInput")
    xkT = nc.dram_tensor("xkT", [128, NK, S], BF, kind="ExternalInput")
    xvT = nc.dram_tensor("xvT", [128, NK, S], BF, kind="ExternalInput")
    wqT = nc.dram_tensor("wqT", [128, NK, GS], BF, kind="ExternalInput")
    wkT = nc.dram_tensor("wkT", [128, NK, GS], BF, kind="ExternalInput")
    wvT = nc.dram_tensor("wvT", [128, NK, GS], BF, kind="ExternalInput")
    woT = nc.dram_tensor("woT", [128, NP, D], BF, kind="ExternalInput")
    biasqk = nc.dram_tensor("biasqk", [128, 8], F32, kind="ExternalInput")
    outTs = [nc.dram_tensor(f"outT{p}", [128, NK, S], F16,
                            kind="ExternalOutput") for p in range(2)]

    with tile.TileContext(nc) as tc, ExitStack() as kctx:
        consts = kctx.enter_context(tc.tile_pool(name="consts", bufs=1))
        pool_w = kctx.enter_context(tc.tile_pool(name="wp", bufs=1))
        pool_xq = kctx.enter_context(tc.tile_pool(name="xqp", bufs=1))
        pool_xs = kctx.enter_context(tc.tile_pool(name="xsp", bufs=3))
        pool_k = kctx.enter_context(tc.tile_pool(name="kTp", bufs=1))
        pool_q = kctx.enter_context(tc.tile_pool(name="qTp", bufs=4))
        pool_va = kctx.enter_context(tc.tile_pool(name="vap", bufs=1))
        pool_e = kctx.enter_context(tc.tile_pool(name="ep", bufs=8))
        pool_oT = kctx.enter_context(tc.tile_pool(name="oTp", bufs=1))
        pool_rr = kctx.enter_context(tc.tile_pool(name="rrp", bufs=3))
        pool_rb = kctx.enter_context(tc.tile_pool(name="rbp", bufs=3))
        pool_oo = kctx.enter_context(tc.tile_pool(name="oop", bufs=8))
        pool_ob = kctx.enter_context(tc.tile_pool(name="obp", bufs=2))
        pp_qk = kctx.enter_context(tc.tile_pool(name="ppqk", bufs=2,
                                                space="PSUM"))
        pp_av = kctx.enter_context(tc.tile_pool(name="ppav", bufs=2,
                                                space="PSUM"))
        pp_fill = kctx.enter_context(tc.tile_pool(name="ppfl", bufs=2,
                                                  space="PSUM"))

        bias_t = consts.tile([128, 8], F32)

        # ---------------- static SBUF tensors ----------------
        wk_t = pool_w.tile([128, NK, GS], BF, name="wk")
        wq_t = pool_w.tile([128, NK, GS], BF, name="wq")
        wv_t = pool_w.tile([128, NK, GS], BF, name="wv")
        wo_t = pool_w.tile([128, NP, D], BF, name="wo")
        xq_t = pool_xq.tile([128, NK, S], BF, name="xq")
        kT = [pool_k.tile([128, S], BF, name=f"kT{m}") for m in range(NP)]
        v_aug = [pool_va.tile([128, 8, HD + 1], BF, name=f"va{i}")
                 for i in range(NSK)]
        o_tiles = [pool_oT.tile([128, S], BF, name=f"oT{a}")
                   for a in range(NP)]
        q_tiles = {}

        # ---------------- prologue: k-proj (all pairs) ----------------
        nc.sync.dma_start(out=wk_t[:, 0:4, 0:256], in_=wkT[:, 0:4, 0:256])
        xk_tiles = {}

        def xk_dma(n, split=False):
            xk_tiles[n] = pool_xs.tile([128, NK, 512], BF, tag="xs",
                                       name=f"xk{n}")
            if split:
                for q in range(4):
                    nc.sync.dma_start(
                        out=xk_tiles[n][:, 2 * q:2 * q + 2, :],
                        in_=xkT[:, 2 * q:2 * q + 2,
                                n * 512:(n + 1) * 512])
            else:
                nc.sync.dma_start(out=xk_tiles[n],
                                  in_=xkT[:, :, n * 512:(n + 1) * 512])

        def kproj_group(m, n, tiles):
            ps = pp_fill.tile([128, 512], F32, tag="fl", name=f"psk{n}{m}")
            for kk in range(NK):
                nc.tensor.matmul(
                    ps[:],
                    wk_t[:, kk, m * 128:(m + 1) * 128],
                    tiles[n][:, kk, :],
                    start=(kk == 0),
                    stop=(kk == NK - 1),
                )
            nc.vector.tensor_scalar_add(
                kT[m][:, n * 512:(n + 1) * 512], ps[:],
                bias_t[:, 4 + m:5 + m])

        # prologue covers pairs 0-2; pair 3 runs as mid-span fillers
        xk_dma(0, split=True)
        nc.sync.dma_start(out=wk_t[:, 4:8, 0:256], in_=wkT[:, 4:8, 0:256])
        nc.sync.dma_start(out=bias_t, in_=biasqk[:, :])
        nc.sync.dma_start(out=wk_t[:, :, 256:512], in_=wkT[:, :, 256:512])
        xk_dma(1)
        for n in range(NCH):
            if n + 2 < NCH:
                xk_dma(n + 2)
            if n == 2:
                nc.sync.dma_start(out=wq_t, in_=wqT[:, :, :])
            if n == 3:
                nc.sync.dma_start(out=wv_t, in_=wvT[:, :, :])
            for m in range(3):
                kproj_group(m, n, xk_tiles)

        def kproj_fillers(m):
            """4 filler groups for k-proj of pair m (re-streams xk)."""
            tiles = {}

            def dma(n):
                tiles[n] = pool_xs.tile([128, NK, 512], BF, tag="xs",
                                        name=f"xk{m}_{n}")
                nc.sync.dma_start(out=tiles[n],
                                  in_=xkT[:, :, n * 512:(n + 1) * 512])

            def group(n):
                def run():
                    if n + 1 < NCH:
                        dma(n + 1)
                    kproj_group(m, n, tiles)
                return run
            return [group(n) for n in range(NCH)], dma

        def xq_dma(n):
            nc.sync.dma_start(out=xq_t[:, :, n * 512:(n + 1) * 512],
                              in_=xqT[:, :, n * 512:(n + 1) * 512])

        def qproj_groups(a):
            qt = pool_q.tile([128, S], BF, tag="qT", name=f"qT{a}")
            q_tiles[a] = qt

            def group(n):
                def run():
                    ps = pp_fill.tile([128, 512], F32, tag="fl",
                                     name=f"psq{a}{n}")
                    for kk in range(NK):
                        nc.tensor.matmul(
                            ps[:],
                            wq_t[:, kk, a * 128:(a + 1) * 128],
                            xq_t[:, kk, n * 512:(n + 1) * 512],
                            start=(kk == 0),
                            stop=(kk == NK - 1),
                        )
                    nc.vector.tensor_scalar_add(
                        qt[:, n * 512:(n + 1) * 512], ps[:],
                        bias_t[:, a:a + 1])
                return run
            return [group(n) for n in range(NCH)]

        q0 = qproj_groups(0)

        # ---------------- vT-proj groups (one per seq-tile st) ----------
        xv_tiles = {}

        def xv_dma(n):
            xv_tiles[n] = pool_xs.tile([128, NK, 512], BF, tag="xs",
                                       name=f"xv{n}")
            nc.sync.dma_start(out=xv_tiles[n],
                              in_=xvT[:, :, n * 512:(n + 1) * 512])

        def vt_group(st):
            def run():
                n, sl = st // 4, st % 4
                if sl == 0 and 1 <= n < NCH - 1:
                    xv_dma(n + 1)
                ps = pp_fill.tile([128, 512], F32, tag="fl", name=f"psv{st}")
                for kk in range(NK):
                    nc.tensor.matmul(
                        ps[:],
                        xv_tiles[n][:, kk, sl * 128:(sl + 1) * 128],
                        wv_t[:, kk, :],
                        start=(kk == 0),
                        stop=(kk == NK - 1),
                    )
                nc.vector.memset(v_aug[st][:, :, HD:HD + 1], 1.0)
                nc.vector.tensor_copy(v_aug[st][:, :, 0:HD], ps[:])
            return run

        vt_fill = [vt_group(st) for st in range(NSK)]
        xq_dma(0)
        xv_dma(0)
        q0[0]()
        xq_dma(1)
        vt_fill[0]()
        xv_dma(1)
        vt_fill[1]()
        xq_dma(2)
        xq_dma(3)

        nc.sync.dma_start(out=wo_t, in_=woT[:, :, :])

        # ---------------- out-proj groups ----------------
        def outproj_groups(p, jjs, copy_eng="vector", pool_alt=False):
            def group(dm, jj, gi):
                def run():
                    if pool_alt and gi % 2:
                        # post-exp groups: the score ring is idle, borrow its
                        # slots to double the effective psum ring depth
                        ps = pp_qk.tile([128, 512], F32, tag="sc",
                                        name=f"pso{p}{dm}{jj}")
                    else:
                        ps = pp_fill.tile([128, 512], F32, tag="fl",
                                         name=f"pso{p}{dm}{jj}")
                    for a in (2 * p, 2 * p + 1):
                        nc.tensor.matmul(
                            ps[:],
                            wo_t[:, a, dm * 128:(dm + 1) * 128],
                            o_tiles[a][:, jj * 512:(jj + 1) * 512],
                            start=(a == 2 * p),
                            stop=(a == 2 * p + 1),
                        )
                    oo = pool_oo.tile([128, 512], F16, tag="oo",
                                      name=f"oo{p}{dm}{jj}")
                    use_act = (copy_eng == "scalar"
                               or (copy_eng == "alt" and gi % 2))
                    with nc.allow_low_precision(reason="fp16 partial out"):
                        if use_act:
                            nc.scalar.copy(oo[:], ps[:])
                        else:
                            nc.vector.tensor_copy(oo[:], ps[:])
                    nc.sync.dma_start(
                        out=outTs[p][:, dm, jj * 512:(jj + 1) * 512],
                        in_=oo[:])
                return run
            return [group(dm, jj, gi)
                    for gi, (jj, dm) in enumerate(
                        (jj, dm) for jj in jjs for dm in range(NK))]

        # ---------------- attention ----------------
        def make_qk_exp(a, j, ee):
            def qk_exp(p):
                for h in range(2):
                    hb = h * HD
                    sc = pp_qk.tile([128, 2, 512], F32, tag="sc",
                                    name=f"sc{a}{j}{p}{h}")
                    for u in range(2):
                        i = 2 * p + u
                        nc.tensor.matmul(
                            sc[:, u, :],
                            kT[a][hb:hb + HD, i * 128:(i + 1) * 128],
                            q_tiles[a][hb:hb + HD,
                                       j * SQ:(j + 1) * SQ],
                            start=True,
                            stop=True,
                        )
                    e = pool_e.tile([128, 2, 512], BF, tag="e",
                                    name=f"e{a}{j}{p}{h}")
                    nc.scalar.activation(e[:], sc[:], AF.Exp)
                    ee[(p, h)] = e
            return qk_exp

        def attn_block(a, j, fillers, pre, nxt, pace_off=1,
                       tail_fillers=()):
            """Depth-2 software pipeline over key-tile PAIRS: step p emits
            [fillers, QK/exp(p+2), AV(p)]. Scores for (2p, 2p+1) share one
            [128,2,512] psum tile per head so exp stays at 1024-wide; filler
            matmuls use their own pp_fill pool and never touch the score
            ring. The last two QK slots prefetch the next block."""
            po = [pp_av.tile([HD + 1, SQ], F32, tag="po",
                             name=f"po{a}{j}{h}") for h in range(2)]
            ee = pre if pre is not None else {}
            qk_exp = make_qk_exp(a, j, ee)
            if pre is None:
                qk_exp(0)
                qk_exp(1)
            nee = {}
            nqk = make_qk_exp(nxt[0], nxt[1], nee) if nxt else None
            nf, fi = len(fillers), 0
            for p in range(NIP):
                want = (p + pace_off) * nf // NIP
                while fi < min(want, nf):
                    fillers[fi]()
                    fi += 1
                if p + 2 < NIP:
                    qk_exp(p + 2)
                elif nqk is not None:
                    nqk(p + 2 - NIP)
                for h in range(2):
                    e = ee.pop((p, h))
                    for u in range(2):
                        i = 2 * p + u
                        nc.tensor.matmul(
                            po[h][:],
                            v_aug[i][:, 2 * a + h, :],
                            e[:, u, :],
                            start=(i == 0),
                            stop=(i == NSK - 1),
                        )
            while fi < nf:
                fillers[fi]()
                fi += 1
            for g in tail_fillers:
                g()
            # normalization: 1/rowsum broadcast (Pool) and multiply (DVE);
            # recips first so the h0/h1 chains overlap across engines
            with nc.allow_low_precision(reason="bf16 softmax reciprocal"):
                rrs, rbs = [], []
                for h in range(2):
                    rr = pool_rr.tile([1, SQ], BF, tag="rr",
                                      name=f"rr{a}{j}{h}")
                    nc.vector.reciprocal(rr[:], po[h][HD:HD + 1, :])
                    rrs.append(rr)
                for h in range(2):
                    rb = pool_rb.tile([HD, SQ], BF, tag="rb",
                                      name=f"rb{a}{j}{h}")
                    nc.gpsimd.partition_broadcast(rb[:], rrs[h][:])
                    rbs.append(rb)
                for h in range(2):
                    hb = h * HD
                    nc.vector.tensor_mul(
                        o_tiles[a][hb:hb + HD, j * SQ:(j + 1) * SQ],
                        po[h][0:HD, :],
                        rbs[h][:])
            return nee if nxt else None

        q1 = qproj_groups(1)
        q2 = qproj_groups(2)
        q3 = qproj_groups(3)
        k3, k3_dma = kproj_fillers(3)
        op01 = outproj_groups(0, (0, 1, 2, 3))
        op23_0 = outproj_groups(1, (0,))
        op23_1 = outproj_groups(1, (1,))
        def merged_op23(jj):
            """out-proj pairs 2-3 for query chunk jj, staged into one tile
            and shipped with a single DMA (one HWDGE fixed cost)."""
            ob = pool_ob.tile([128, NK, 512], F16, tag="ob",
                              name=f"ob{jj}")

            def unit(dm):
                def run():
                    if dm % 2:
                        ps = pp_qk.tile([128, 512], F32, tag="sc",
                                        name=f"pst{jj}{dm}")
                    else:
                        ps = pp_fill.tile([128, 512], F32, tag="fl",
                                         name=f"pst{jj}{dm}")
                    for a in (2, 3):
                        nc.tensor.matmul(
                            ps[:],
                            wo_t[:, a, dm * 128:(dm + 1) * 128],
                            o_tiles[a][:, jj * 512:(jj + 1) * 512],
                            start=(a == 2),
                            stop=(a == 3),
                        )
                    with nc.allow_low_precision(reason="fp16 partial out"):
                        if dm % 2:
                            nc.scalar.copy(ob[:, dm, :], ps[:])
                        else:
                            nc.vector.tensor_copy(ob[:, dm, :], ps[:])
                return run

            def ship():
                nc.sync.dma_start(
                    out=outTs[1][:, :, jj * 512:(jj + 1) * 512], in_=ob[:])
            return [unit(dm) for dm in range(NK)] + [ship]

        op23_2t = merged_op23(2)

        plan = [
            (0, 0, vt_fill[2:] + [q0[1]], 3, ()),
            (0, 1, [q0[2], q1[0]], 1, ()),
            (0, 2, [q0[3], q1[1]], 1, ()),
            (0, 3, [q1[2], q1[3], lambda: k3_dma(0)], 1, ()),
            (1, 0, [k3[0], q2[0]], 1, ()),
            (1, 1, [k3[1], q2[1]], 1, ()),
            (1, 2, [k3[2], q2[2]], 1, ()),
            (1, 3, [k3[3], q2[3]], 1, ()),
            (2, 0, [q3[0]] + op01[0:4], 1, ()),
            (2, 1, [q3[1]] + op01[4:8], 1, ()),
            (2, 2, [q3[2]] + op01[8:12], 1, ()),
            (2, 3, [q3[3]] + op01[12:16], 1, ()),
            (3, 0, op01[16:24], 1, ()),
            (3, 1, op01[24:32], 1, ()),
            (3, 2, op23_0, 1, ()),
            # tail fillers (alt copies) hide the final norm chain
            (3, 3, op23_1, 1, op23_2t),
        ]
        pre = None
        for bi, (a, j, fillers, off, tails) in enumerate(plan):
            nxt = plan[bi + 1][0:2] if bi + 1 < len(plan) else None
            pre = attn_block(a, j, fillers, pre, nxt, pace_off=off,
                             tail_fillers=tails)
        # epilogue (out-proj pairs 2-3, jj=3): same merged-DMA shape
        for g in merged_op23(3):
            g()

    nc.compile()
    _CACHE["nc"] = nc
    return nc


def _tox(a):
    """[1024|512, N] -> [128, k, N] bf16 (partition-major k-tiling)."""
    r = a.shape[0] // 128
    return np.ascontiguousarray(
        a.reshape(r, 128, a.shape[1]).transpose(1, 0, 2)).astype(NPBF)


def kernel(Q, K, V, Wq, bq, Wk, bk, Wv, bv, Wo, bo):
    Q = np.asarray(Q, np.float32)
    K = np.asarray(K, np.float32)
    V = np.asarray(V, np.float32)
    Wq = np.asarray(Wq, np.float32)
    Wk = np.asarray(Wk, np.float32)
    Wv = np.asarray(Wv, np.float32)
    Wo = np.asarray(Wo, np.float32)
    bq = np.asarray(bq, np.float32)
    bk = np.asarray(bk, np.float32)
    bv = np.asarray(bv, np.float32)
    bo = np.asarray(bo, np.float32)
    scale = 1.0 / 8.0  # 1/sqrt(HD), folded into the q projection

    nc = _build_nc()
    in_maps = []
    for c in range(8):
        b, g = divmod(c, 2)
        gs = slice(g * GS, (g + 1) * GS)
        biasqk = np.empty((128, 8), np.float32)
        for m in range(NP):
            biasqk[:, m] = bq[gs][m * 128:(m + 1) * 128] * scale
            biasqk[:, 4 + m] = bk[gs][m * 128:(m + 1) * 128]
        in_maps.append({
            "xqT": _tox(Q[b].T),
            "xkT": _tox(K[b].T),
            "xvT": _tox(V[b].T),
            "wqT": _tox((Wq[gs] * scale).T),
            "wkT": _tox(Wk[gs].T),
            "wvT": _tox(Wv[gs].T),
            "woT": _tox(Wo[:, gs].T),
            "biasqk": biasqk,
        })

    host_bias = bo + Wo @ bv  # v bias folded through softmax + out-proj

    def run_and_gather():
        res = run_bass_kernel_spmd(nc, in_maps, list(range(8)))
        out = np.empty((B, S, D), np.float32)
        for b in range(B):
            acc = None
            for c in (2 * b, 2 * b + 1):
                for p in range(2):
                    part = np.asarray(res.results[c][f"outT{p}"])
                    part = part.transpose(1, 0, 2).reshape(D, S)
                    acc = part if acc is None else acc + part
            out[b] = acc.T + host_bias
        return out

    try:
        return run_and_gather()
    except Exception:
        # transient device wedge (e.g. NRT_EXEC_UNIT_UNRECOVERABLE) can
        # surface either in the run or in result materialization: retry once
        return run_and_gather()


# revision 25
# speedup vs baseline: 1.0162x; 1.0063x over previous
"""Multi-head attention (B=4, S=2048, D=1024, H=16) on 8 TRN2 NeuronCores.

Sharding: core c -> (batch b = c//2, head-group g = c%2): each core runs 8
heads of one batch (dout slice of 512) and emits two fp16 out-projection
partials (pairs 0-1 and 2-3); the host sums 4 partials per batch + bias.

All matmul operands are bf16 (fp32 PSUM accumulation); exp runs on the Act
engine (fp32 psum -> bf16); the softmax row-sum is folded into the AV matmul
via a ones-column on v; normalization = DVE reciprocal -> GPSIMD
partition_broadcast -> DVE multiply (no PE involvement). v-projection is
computed directly in transposed [seq, dout] layout (no PE transposes). The
v bias is folded into the host-side output bias (softmax rows sum to 1).

Schedule: k-proj (pairs 0-2) + q-proj(pair0, chunk0) prologue with
interleaved DMA sequencing; 16 attention blocks (pair a, 512-wide query
chunk j) run a depth-2 software pipeline over key-tile PAIRS (step p emits
fillers, QK/exp(p+2), AV(p)); scores for (2p, 2p+1) share one [128,2,512]
psum tile per head so exp stays 1024-wide. PSUM: score ring 2x[128,2,512]
(4 banks) + AV accumulators 2x[65,512] (2 banks) + a DEDICATED filler pool
2x[128,512] (2 banks) — filler matmuls (vT-proj, q/k-proj, out-proj) never
touch the score ring, which keeps its slot-reuse dependencies phase-locked
to one-period-old exps. The last two QK slots of each block prefetch the
next block; out-proj groups depending on the final normalization run as
tail fillers (alt-engine copies hide the norm chain) and a short epilogue.
Out-projection partials are written as fp16 (halves the output DMA). PE is
the critical engine at ~94.5% busy (~331 us of 351 us total); Act/exp
~272 us, DVE ~112 us, DMA ~84 us, Pool ~26 us all hide under it.
"""
from contextlib import ExitStack

import ml_dtypes
import numpy as np

import concourse.bacc as bacc
import concourse.tile as tile
from concourse import mybir
from concourse.bass_utils import run_bass_kernel_spmd

F32 = mybir.dt.float32
F16 = mybir.dt.float16
ALU_ADD = mybir.AluOpType.add
BF = mybir.dt.bfloat16
AF = mybir.ActivationFunctionType
NPBF = ml_dtypes.bfloat16

B, S, D, H, HD = 4, 2048, 1024, 16, 64
GS = D // 2            # 512: per-core dout slice (8 heads, 4 pairs)
NP = GS // 128         # 4 head pairs (= dout tiles = wo k-tiles)
NK = D // 128          # 8 din k-tiles
NSK = S // 128         # 16 key tiles
SQ = 512               # query chunk (block width)
NSQ = S // SQ          # 4
NIP = NSK // 2         # 8 key-tile pairs per block
NCH = S // 512         # 4 (512-wide chunks of S)

_CACHE = {}


def _build_nc():
    if "nc" in _CACHE:
        return _CACHE["nc"]

    nc = bacc.Bacc()

    xqT = nc.dram_tensor("xqT", [128, NK, S], BF, kind="ExternalInput")
    xkT = nc.dram_tensor("xkT", [128, NK, S], BF, kind="ExternalInput")
    xvT = nc.dram_tensor("xvT", [128, NK, S], BF, kind="ExternalInput")
    wqT = nc.dram_tensor("wqT", [128, NK, GS], BF, kind="ExternalInput")
    wkT = nc.dram_tensor("wkT", [128, NK, GS], BF, kind="ExternalInput")
    wvT = nc.dram_tensor("wvT", [128, NK, GS], BF, kind="ExternalInput")
    woT = nc.dram_tensor("woT", [128, NP, D], BF, kind="ExternalInput")
    biasqk = nc.dram_tensor("biasqk", [128, 8], F32, kind="ExternalInput")
    outTs = [nc.dram_tensor(f"outT{p}", [128, NK, S], F16,
                            kind="ExternalOutput") for p in range(2)]

    with tile.TileContext(nc) as tc, ExitStack() as kctx:
        consts = kctx.enter_context(tc.tile_pool(name="consts", bufs=1))
        pool_w = kctx.enter_context(tc.tile_pool(name="wp", bufs=1))
        pool_xq = kctx.enter_context(tc.tile_pool(name="xqp", bufs=1))
        pool_xs = kctx.enter_context(tc.tile_pool(name="xsp", bufs=3))
        pool_k = kctx.enter_context(tc.tile_pool(name="kTp", bufs=1))
        pool_q = kctx.enter_context(tc.tile_pool(name="qTp", bufs=4))
        pool_va = kctx.enter_context(tc.tile_pool(name="vap", bufs=1))
        pool_e = kctx.enter_context(tc.tile_pool(name="ep", bufs=8))
        pool_oT = kctx.enter_context(tc.tile_pool(name="oTp", bufs=1))
        pool_rr = kctx.enter_context(tc.tile_pool(name="rrp", bufs=3))
        pool_rb = kctx.enter_context(tc.tile_pool(name="rbp", bufs=3))
        pool_oo = kctx.enter_context(tc.tile_pool(name="oop", bufs=8))
        pool_ob = kctx.enter_context(tc.tile_pool(name="obp", bufs=2))
        pp_qk = kctx.enter_context(tc.tile_pool(name="ppqk", bufs=2,
                                                space="PSUM"))
        pp_av = kctx.enter_context(tc.tile_pool(name="ppav", bufs=2,
                                                space="PSUM"))
        pp_fill = kctx.enter_context(tc.tile_pool(name="ppfl", bufs=2,
                                                  space="PSUM"))

        bias_t = consts.tile([128, 8], F32)

        # ---------------- static SBUF tensors ----------------
        wk_t = pool_w.tile([128, NK, GS], BF, name="wk")
        wq_t = pool_w.tile([128, NK, GS], BF, name="wq")
        wv_t = pool_w.tile([128, NK, GS], BF, name="wv")
        wo_t = pool_w.tile([128, NP, D], BF, name="wo")
        xq_t = pool_xq.tile([128, NK, S], BF, name="xq")
        kT = [pool_k.tile([128, S], BF, name=f"kT{m}") for m in range(NP)]
        v_aug = [pool_va.tile([128, 8, HD + 1], BF, name=f"va{i}")
                 for i in range(NSK)]
        o_tiles = [pool_oT.tile([128, S], BF, name=f"oT{a}")
                   for a in range(NP)]
        q_tiles = {}

        # ---------------- prologue: k-proj (all pairs) ----------------
        nc.sync.dma_start(out=wk_t[:, 0:4, 0:256], in_=wkT[:, 0:4, 0:256])
        xk_tiles = {}

        def xk_dma(n, split=False):
            xk_tiles[n] = pool_xs.tile([128, NK, 512], BF, tag="xs",
                                       name=f"xk{n}")
            if split:
                for q in range(4):
                    nc.sync.dma_start(
                        out=xk_tiles[n][:, 2 * q:2 * q + 2, :],
                        in_=xkT[:, 2 * q:2 * q + 2,
                                n * 512:(n + 1) * 512])
            else:
                nc.sync.dma_start(out=xk_tiles[n],
                                  in_=xkT[:, :, n * 512:(n + 1) * 512])

        def kproj_group(m, n, tiles):
            ps = pp_fill.tile([128, 512], F32, tag="fl", name=f"psk{n}{m}")
            for kk in range(NK):
                nc.tensor.matmul(
                    ps[:],
                    wk_t[:, kk, m * 128:(m + 1) * 128],
                    tiles[n][:, kk, :],
                    start=(kk == 0),
                    stop=(kk == NK - 1),
                )
            nc.vector.tensor_scalar_add(
                kT[m][:, n * 512:(n + 1) * 512], ps[:],
                bias_t[:, 4 + m:5 + m])

        # prologue covers pairs 0-2; pair 3 runs as mid-span fillers
        xk_dma(0, split=True)
        nc.sync.dma_start(out=wk_t[:, 4:8, 0:256], in_=wkT[:, 4:8, 0:256])
        nc.sync.dma_start(out=bias_t, in_=biasqk[:, :])
        nc.sync.dma_start(out=wk_t[:, :, 256:512], in_=wkT[:, :, 256:512])
        xk_dma(1)
        for n in range(NCH):
            if n + 2 < NCH:
                xk_dma(n + 2)
            if n == 2:
                nc.sync.dma_start(out=wq_t, in_=wqT[:, :, :])
            if n == 3:
                nc.sync.dma_start(out=wv_t, in_=wvT[:, :, :])
            for m in range(3):
                kproj_group(m, n, xk_tiles)

        def kproj_fillers(m):
            """4 filler groups for k-proj of pair m (re-streams xk)."""
            tiles = {}

            def dma(n):
                tiles[n] = pool_xs.tile([128, NK, 512], BF, tag="xs",
                                        name=f"xk{m}_{n}")
                nc.sync.dma_start(out=tiles[n],
                                  in_=xkT[:, :, n * 512:(n + 1) * 512])

            def group(n):
                def run():
                    if n + 1 < NCH:
                        dma(n + 1)
                    kproj_group(m, n, tiles)
                return run
            return [group(n) for n in range(NCH)], dma

        def xq_dma(n):
            nc.sync.dma_start(out=xq_t[:, :, n * 512:(n + 1) * 512],
                              in_=xqT[:, :, n * 512:(n + 1) * 512])

        def qproj_groups(a):
            qt = pool_q.tile([128, S], BF, tag="qT", name=f"qT{a}")
            q_tiles[a] = qt

            def group(n):
                def run():
                    ps = pp_fill.tile([128, 512], F32, tag="fl",
                                     name=f"psq{a}{n}")
                    for kk in range(NK):
                        nc.tensor.matmul(
                            ps[:],
                            wq_t[:, kk, a * 128:(a + 1) * 128],
                            xq_t[:, kk, n * 512:(n + 1) * 512],
                            start=(kk == 0),
                            stop=(kk == NK - 1),
                        )
                    nc.vector.tensor_scalar_add(
                        qt[:, n * 512:(n + 1) * 512], ps[:],
                        bias_t[:, a:a + 1])
                return run
            return [group(n) for n in range(NCH)]

        q0 = qproj_groups(0)

        # ---------------- vT-proj groups (one per seq-tile st) ----------
        xv_tiles = {}

        def xv_dma(n):
            xv_tiles[n] = pool_xs.tile([128, NK, 512], BF, tag="xs",
                                       name=f"xv{n}")
            nc.sync.dma_start(out=xv_tiles[n],
                              in_=xvT[:, :, n * 512:(n + 1) * 512])

        def vt_group(st):
            def run():
                n, sl = st // 4, st % 4
                if sl == 0 and 1 <= n < NCH - 1:
                    xv_dma(n + 1)
                ps = pp_fill.tile([128, 512], F32, tag="fl", name=f"psv{st}")
                for kk in range(NK):
                    nc.tensor.matmul(
                        ps[:],
                        xv_tiles[n][:, kk, sl * 128:(sl + 1) * 128],
                        wv_t[:, kk, :],
                        start=(kk == 0),
                        stop=(kk == NK - 1),
                    )
                nc.vector.memset(v_aug[st][:, :, HD:HD + 1], 1.0)
                nc.vector.tensor_copy(v_aug[st][:, :, 0:HD], ps[:])
            return run

        vt_fill = [vt_group(st) for st in range(NSK)]
        xq_dma(0)
        xv_dma(0)
        q0[0]()
        xq_dma(1)
        vt_fill[0]()
        xv_dma(1)
        vt_fill[1]()
        xq_dma(2)
        xq_dma(3)

        nc.sync.dma_start(out=wo_t, in_=woT[:, :, :])

        # ---------------- out-proj groups ----------------
        def outproj_groups(p, jjs, copy_eng="vector", pool_alt=False):
            def group(dm, jj, gi):
                def run():
                    if pool_alt and gi % 2:
                        # post-exp groups: the score ring is idle, borrow its
                        # slots to double the effective psum ring depth
                        ps = pp_qk.tile([128, 512], F32, tag="sc",
                                        name=f"pso{p}{dm}{jj}")
                    else:
                        ps = pp_fill.tile([128, 512], F32, tag="fl",
                                         name=f"pso{p}{dm}{jj}")
                    for a in (2 * p, 2 * p + 1):
                        nc.tensor.matmul(
                            ps[:],
                            wo_t[:, a, dm * 128:(dm + 1) * 128],
                            o_tiles[a][:, jj * 512:(jj + 1) * 512],
                            start=(a == 2 * p),
                            stop=(a == 2 * p + 1),
                        )
                    oo = pool_oo.tile([128, 512], F16, tag="oo",
                                      name=f"oo{p}{dm}{jj}")
                    use_act = (copy_eng == "scalar"
                               or (copy_eng == "alt" and gi % 2))
                    with nc.allow_low_precision(reason="fp16 partial out"):
                        if use_act:
                            nc.scalar.copy(oo[:], ps[:])
                        else:
                            nc.vector.tensor_copy(oo[:], ps[:])
                    nc.sync.dma_start(
                        out=outTs[p][:, dm, jj * 512:(jj + 1) * 512],
                        in_=oo[:])
                return run
            return [group(dm, jj, gi)
                    for gi, (jj, dm) in enumerate(
                        (jj, dm) for jj in jjs for dm in range(NK))]

        # ---------------- attention ----------------
        def make_qk_exp(a, j, ee):
            def qk_exp(p):
                for h in range(2):
                    hb = h * HD
                    sc = pp_qk.tile([128, 2, 512], F32, tag="sc",
                                    name=f"sc{a}{j}{p}{h}")
                    for u in range(2):
                        i = 2 * p + u
                        nc.tensor.matmul(
                            sc[:, u, :],
                            kT[a][hb:hb + HD, i * 128:(i + 1) * 128],
                            q_tiles[a][hb:hb + HD,
                                       j * SQ:(j + 1) * SQ],
                            start=True,
                            stop=True,
                        )
                    e = pool_e.tile([128, 2, 512], BF, tag="e",
                                    name=f"e{a}{j}{p}{h}")
                    nc.scalar.activation(e[:], sc[:], AF.Exp)
                    ee[(p, h)] = e
            return qk_exp

        def attn_block(a, j, fillers, pre, nxt, pace_off=1,
                       tail_fillers=()):
            """Depth-2 software pipeline over key-tile PAIRS: step p emits
            [fillers, QK/exp(p+2), AV(p)]. Scores for (2p, 2p+1) share one
            [128,2,512] psum tile per head so exp stays at 1024-wide; filler
            matmuls use their own pp_fill pool and never touch the score
            ring. The last two QK slots prefetch the next block."""
            po = [pp_av.tile([HD + 1, SQ], F32, tag="po",
                             name=f"po{a}{j}{h}") for h in range(2)]
            ee = pre if pre is not None else {}
            qk_exp = make_qk_exp(a, j, ee)
            if pre is None:
                qk_exp(0)
                qk_exp(1)
            nee = {}
            nqk = make_qk_exp(nxt[0], nxt[1], nee) if nxt else None
            nf, fi = len(fillers), 0
            for p in range(NIP):
                want = (p + pace_off) * nf // NIP
                while fi < min(want, nf):
                    fillers[fi]()
                    fi += 1
                if p + 2 < NIP:
                    qk_exp(p + 2)
                elif nqk is not None:
                    nqk(p + 2 - NIP)
                for h in range(2):
                    e = ee.pop((p, h))
                    for u in range(2):
                        i = 2 * p + u
                        nc.tensor.matmul(
                            po[h][:],
                            v_aug[i][:, 2 * a + h, :],
                            e[:, u, :],
                            start=(i == 0),
                            stop=(i == NSK - 1),
                        )
            while fi < nf:
                fillers[fi]()
                fi += 1
            for g in tail_fillers:
                g()
            # normalization: 1/rowsum broadcast (Pool) and multiply (DVE);
            # recips first so the h0/h1 chains overlap across engines
            with nc.allow_low_precision(reason="bf16 softmax reciprocal"):
                rrs, rbs = [], []
                for h in range(2):
                    rr = pool_rr.tile([1, SQ], BF, tag="rr",
                                      name=f"rr{a}{j}{h}")
                    nc.vector.reciprocal(rr[:], po[h][HD:HD + 1, :])
                    rrs.append(rr)
                for h in range(2):
                    rb = pool_rb.tile([HD, SQ], BF, tag="rb",
                                      name=f"rb{a}{j}{h}")
                    nc.gpsimd.partition_broadcast(rb[:], rrs[h][:])
                    rbs.append(rb)
                for h in range(2):
                    hb = h * HD
                    nc.vector.tensor_mul(
                        o_tiles[a][hb:hb + HD, j * SQ:(j + 1) * SQ],
                        po[h][0:HD, :],
                        rbs[h][:])
            return nee if nxt else None

        q1 = qproj_groups(1)
        q2 = qproj_groups(2)
        q3 = qproj_groups(3)
        k3, k3_dma = kproj_fillers(3)
        op01 = outproj_groups(0, (0, 1, 2, 3))
        op23_0 = outproj_groups(1, (0,))
        op23_1 = outproj_groups(1, (1,))
        def merged_op23(jj):
            """out-proj pairs 2-3 for query chunk jj, staged into one tile
            and shipped with a single DMA (one HWDGE fixed cost)."""
            ob = pool_ob.tile([128, NK, 512], F16, tag="ob",
                              name=f"ob{jj}")

            def unit(dm):
                def run():
                    if dm % 2:
                        ps = pp_qk.tile([128, 512], F32, tag="sc",
                                        name=f"pst{jj}{dm}")
                    else:
                        ps = pp_fill.tile([128, 512], F32, tag="fl",
                                         name=f"pst{jj}{dm}")
                    for a in (2, 3):
                        nc.tensor.matmul(
                            ps[:],
                            wo_t[:, a, dm * 128:(dm + 1) * 128],
                            o_tiles[a][:, jj * 512:(jj + 1) * 512],
                            start=(a == 2),
                            stop=(a == 3),
                        )
                    with nc.allow_low_precision(reason="fp16 partial out"):
                        if dm % 2:
                            nc.scalar.copy(ob[:, dm, :], ps[:])
                        else:
                            nc.vector.tensor_copy(ob[:, dm, :], ps[:])
                return run

            def ship(d0):
                def run():
                    nc.sync.dma_start(
                        out=outTs[1][:, d0:d0 + 2, jj * 512:(jj + 1) * 512],
                        in_=ob[:, d0:d0 + 2, :])
                return run

            # interleave 2-dm ship chunks so transfers overlap the
            # remaining units' matmuls/copies instead of one long final DMA
            seq = []
            for dm in range(NK):
                seq.append(unit(dm))
                if dm % 2:
                    seq.append(ship(dm - 1))
            return seq

        op23_2t = merged_op23(2)

        plan = [
            (0, 0, vt_fill[2:] + [q0[1]], 3, ()),
            (0, 1, [q0[2], q1[0]], 1, ()),
            (0, 2, [q0[3], q1[1]], 1, ()),
            (0, 3, [q1[2], q1[3], lambda: k3_dma(0)], 1, ()),
            (1, 0, [k3[0], q2[0]], 1, ()),
            (1, 1, [k3[1], q2[1]], 1, ()),
            (1, 2, [k3[2], q2[2]], 1, ()),
            (1, 3, [k3[3], q2[3]], 1, ()),
            (2, 0, [q3[0]] + op01[0:4], 1, ()),
            (2, 1, [q3[1]] + op01[4:8], 1, ()),
            (2, 2, [q3[2]] + op01[8:12], 1, ()),
            (2, 3, [q3[3]] + op01[12:16], 1, ()),
            (3, 0, op01[16:24], 1, ()),
            (3, 1, op01[24:32], 1, ()),
            (3, 2, op23_0, 1, ()),
            # tail fillers (alt copies) hide the final norm chain
            (3, 3, op23_1, 1, op23_2t),
        ]
        pre = None
        for bi, (a, j, fillers, off, tails) in enumerate(plan):
            nxt = plan[bi + 1][0:2] if bi + 1 < len(plan) else None
            pre = attn_block(a, j, fillers, pre, nxt, pace_off=off,
                             tail_fillers=tails)
        # epilogue (out-proj pairs 2-3, jj=3): same merged-DMA shape
        for g in merged_op23(3):
            g()

    nc.compile()
    _CACHE["nc"] = nc
    return nc


def _tox(a):
    """[1024|512, N] -> [128, k, N] bf16 (partition-major k-tiling)."""
    r = a.shape[0] // 128
    return np.ascontiguousarray(
        a.reshape(r, 128, a.shape[1]).transpose(1, 0, 2)).astype(NPBF)


def kernel(Q, K, V, Wq, bq, Wk, bk, Wv, bv, Wo, bo):
    Q = np.asarray(Q, np.float32)
    K = np.asarray(K, np.float32)
    V = np.asarray(V, np.float32)
    Wq = np.asarray(Wq, np.float32)
    Wk = np.asarray(Wk, np.float32)
    Wv = np.asarray(Wv, np.float32)
    Wo = np.asarray(Wo, np.float32)
    bq = np.asarray(bq, np.float32)
    bk = np.asarray(bk, np.float32)
    bv = np.asarray(bv, np.float32)
    bo = np.asarray(bo, np.float32)
    scale = 1.0 / 8.0  # 1/sqrt(HD), folded into the q projection

    nc = _build_nc()
    in_maps = []
    for c in range(8):
        b, g = divmod(c, 2)
        gs = slice(g * GS, (g + 1) * GS)
        biasqk = np.empty((128, 8), np.float32)
        for m in range(NP):
            biasqk[:, m] = bq[gs][m * 128:(m + 1) * 128] * scale
            biasqk[:, 4 + m] = bk[gs][m * 128:(m + 1) * 128]
        in_maps.append({
            "xqT": _tox(Q[b].T),
            "xkT": _tox(K[b].T),
            "xvT": _tox(V[b].T),
            "wqT": _tox((Wq[gs] * scale).T),
            "wkT": _tox(Wk[gs].T),
            "wvT": _tox(Wv[gs].T),
            "woT": _tox(Wo[:, gs].T),
            "biasqk": biasqk,
        })

    host_bias = bo + Wo @ bv  # v bias folded through softmax + out-proj

    def run_and_gather():
        res = run_bass_kernel_spmd(nc, in_maps, list(range(8)))
        out = np.empty((B, S, D), np.float32)
        for b in range(B):
            acc = None
            for c in (2 * b, 2 * b + 1):
                for p in range(2):
                    part = np.asarray(res.results[c][f"outT{p}"])
                    part = part.transpose(1, 0, 2).reshape(D, S)
                    acc = part if acc is None else acc + part
            out[b] = acc.T + host_bias
        return out

    try:
        return run_and_gather()
    except Exception:
        # transient device wedge (e.g. NRT_EXEC_UNIT_UNRECOVERABLE) can
        # surface either in the run or in result materialization: retry once
        return run_and_gather()
